# revision 19
# baseline (speedup 1.0000x reference)
"""DMPNN layer on 8 Trainium2 NeuronCores.

Sharding: edges are assigned to the core that owns their *destination* node
(50000 nodes / 8 cores = 6250 each), so the scatter-sum is core-local and no
collectives are needed.  The per-edge source-feature gather is done on the
HOST (same as the edge_attr permutation): each core receives a dense
feature-major stream gx_t = x[src].T for its (padded) edge list, so the
device never runs gpsimd dma_gather — the old bottleneck (~10ns/edge of Q7
descriptor generation, 905us total).

Within a core, edges are grouped by 128-node destination block (scatter-sum
is an accumulating onehot-matmul into one PSUM tile per block).  Per-block
chunk counts are maxed across cores so all 8 cores run the same static
program (SPMD); per-core variation is data only (gx/ea/dest_rel, padded with
dummy edges whose dest_rel=-1 masks them out of the scatter).

Datapath is bf16 (fp32 PSUM accumulation).  The main loop is software-
pipelined with a 2-super skew (mm1 of super s, mm2 of s-1, scatter of s-2)
so the PE never waits on the scalar-engine relus.
"""

import os

# The bass kernel executes through jax's axon/neuron platform.  A stray
# JAX_PLATFORMS=cpu (commonly set to keep jax off neuronxcc) would hide the
# NeuronCores, so drop it before jax is first imported.
if os.environ.get("JAX_PLATFORMS", "").strip() == "cpu":
    os.environ.pop("JAX_PLATFORMS")

import numpy as np

N_NODES = 50000
N_EDGES = 640000
D = 128          # node feature dim == hidden == output dim
EA = 32          # edge attr dim
NC = 8           # cores
NPC = N_NODES // NC   # nodes per core
BLK = 128        # node block width (scatter psum tile)
NB = (NPC + BLK - 1) // BLK   # 49 blocks per core (last one 106 nodes)
CHUNK = 128      # edge chunk (scatter/matmul granularity)
SUPER = 512      # edge super-chunk (mm1/relu batching)
GBATCH = 4096    # edges per DMA batch (1MB bf16 -> near-peak HBM bw)
EPS = 1e-5

F32 = np.float32


def _np_cdt():
    import ml_dtypes
    return ml_dtypes.bfloat16


def _build_schedule(dest: np.ndarray, src: np.ndarray):
    """Group edges by (core, block); pad so the chunk structure is identical
    across cores.  Returns shared schedule + per-core data."""
    core = dest // NPC
    block = (dest % NPC) // BLK

    key = core * NB + block
    order = np.argsort(key, kind="stable")
    key_s = key[order]
    cnt = np.bincount(key, minlength=NC * NB).reshape(NC, NB)

    # shared chunk counts per block: max over cores, >= 1, total a SUPER mult
    n_chunks = np.maximum(1, -(-cnt.max(axis=0) // CHUNK))  # [NB]
    extra = (-int(n_chunks.sum())) % (SUPER // CHUNK)
    n_chunks[NB - 1] += extra
    T_tot = int(n_chunks.sum())
    L_tot = T_tot * CHUNK

    # padded start offset of each block group within a core's stream
    pad_start = np.concatenate([[0], np.cumsum(n_chunks)[:-1]]) * CHUNK

    # rank of each edge within its (core, block) group
    grp_start = np.zeros(NC * NB + 1, np.int64)
    np.cumsum(np.bincount(key, minlength=NC * NB), out=grp_start[1:])
    rank = np.arange(N_EDGES) - grp_start[key_s]

    b_s = key_s % NB
    c_s = key_s // NB
    pos = pad_start[b_s] + rank

    blk_of_chunk = np.repeat(np.arange(NB), n_chunks)
    blk_of_edge = np.repeat(blk_of_chunk, CHUNK)

    per_core = []
    dest_s = dest[order]
    src_s = src[order]
    for c in range(NC):
        m = c_s == c
        p = pos[m]
        src_pad = np.zeros(L_tot, np.int64)
        src_pad[p] = src_s[m]
        dest_rel = np.full(L_tot, -1.0, F32)
        dest_rel[p] = (dest_s[m] % NPC - blk_of_edge[p] * BLK).astype(F32)
        assert dest_rel.max() < BLK and (dest_rel[p] >= 0).all()
        ea_perm = np.full(L_tot, -1, np.int64)
        ea_perm[p] = order[m]   # original edge id per padded slot (-1 = dummy)
        per_core.append(dict(src=src_pad, dest_rel=dest_rel, ea_perm=ea_perm))

    sched = dict(n_chunks=n_chunks, T_tot=T_tot, L_tot=L_tot,
                 blk_of_chunk=blk_of_chunk)
    return sched, per_core


def _build_bass(sched):
    import concourse.bacc as bacc
    import concourse.mybir as mybir
    import concourse.tile as tile

    dt = mybir.dt
    cdt = dt.bfloat16
    T_tot = sched["T_tot"]
    L_tot = sched["L_tot"]
    n_chunks = sched["n_chunks"]
    blk_of_chunk = sched["blk_of_chunk"]
    skip_bias2 = sched["skip_bias2"]
    skip_biasn = sched["skip_biasn"]
    skip_affine = sched["skip_affine"]

    n_sup = T_tot // (SUPER // CHUNK)
    n_batches = -(-L_tot // GBATCH)

    # first/last chunk of each block
    blk_start = np.concatenate([[0], np.cumsum(n_chunks)[:-1]])
    first_of = np.zeros(T_tot, bool)
    last_of = np.zeros(T_tot, bool)
    first_of[blk_start] = True
    last_of[blk_start + n_chunks - 1] = True

    nc = bacc.Bacc("TRN2", target_bir_lowering=False, debug=False,
                   num_devices=NC)

    def din(name, shape, d=None):
        return nc.dram_tensor(name, shape, d or cdt, kind="ExternalInput").ap()

    gx_t = din("gx_t", [D, L_tot])
    ea2 = din("ea2", [2 * EA, L_tot // 2])
    oh_t = din("oh_t", [CHUNK, L_tot], dt.float8e4)  # host-built onehots
    xt_loc = din("xt_loc", [D, NPC])
    x_loc = din("x_loc", [NPC, D], dt.float32)
    w1a = din("w1a", [D, D])
    w1b2 = din("w1b2", [2 * EA, D])   # w1b replicated at partitions 0/32
    w2 = din("w2", [D, D])
    wna = din("wna", [D, D])
    wnb = din("wnb", [D, D])
    b1 = din("b1", [D, 1], dt.float32)
    b2r = din("b2r", [1, D])
    bnr = din("bnr", [1, D])
    ones_r = din("ones_r", [1, 128])
    gma = din("gma", [128, D], dt.float32)
    bta = din("bta", [128, D], dt.float32)
    out = nc.dram_tensor("out", [NPC, D], cdt, kind="ExternalOutput").ap()

    with tile.TileContext(nc) as tc:
        from contextlib import ExitStack
        ctx = ExitStack()
        with ctx:
            const = ctx.enter_context(tc.tile_pool(name="const", bufs=1))
            gpool = ctx.enter_context(tc.tile_pool(name="gx", bufs=2))
            eapool = ctx.enter_context(tc.tile_pool(name="ea", bufs=2))
            hpool = ctx.enter_context(tc.tile_pool(name="h", bufs=3))
            epool = ctx.enter_context(tc.tile_pool(name="eh", bufs=3))
            ohpool = ctx.enter_context(tc.tile_pool(name="ohp", bufs=3))
            psum = ctx.enter_context(tc.tile_pool(name="psum", bufs=2,
                                                  space="PSUM"))
            npool = ctx.enter_context(tc.tile_pool(name="node", bufs=3))

            def load_const(ap, shape, d=None):
                t = const.tile(shape, d or cdt, tag=f"c_{ap.tensor.name}")
                nc.sync.dma_start(out=t[:], in_=ap)
                return t

            w1a_s = load_const(w1a[:], [D, D])
            w1b_s = load_const(w1b2[:], [2 * EA, D])
            w2_s = load_const(w2[:], [D, D])
            if not skip_bias2:
                b2r_s = load_const(b2r[:], [1, D])
            if not (skip_bias2 and skip_biasn):
                ones_s = load_const(ones_r[:], [1, 128])
            if not skip_biasn:
                bnr_s = load_const(bnr[:], [1, D])
            if not skip_affine:
                gma_s = load_const(gma[:], [128, D], dt.float32)
                bta_s = load_const(bta[:], [128, D], dt.float32)

            eps_t = const.tile([128, 1], dt.float32, tag="eps")
            nc.vector.memset(eps_t[:], EPS)

            relu = mybir.ActivationFunctionType.Relu

            def node_mlp(b, an_ps, agg_sb, xb):
                """node MLP + residual layernorm for block b, consuming its
                merged scatter accumulator (SBUF bf16)."""
                n_w = min(BLK, NPC - b * BLK)
                cols = slice(b * BLK, b * BLK + n_w)
                ps_nn = an_ps[:, BLK:BLK + D]
                nc.tensor.matmul(ps_nn[:n_w, :], xt_s[:, cols], wna_s[:],
                                 start=True, stop=False)
                nc.tensor.matmul(ps_nn[:n_w, :], agg_sb[:, :n_w], wnb_s[:],
                                 start=False, stop=skip_biasn)
                if not skip_biasn:
                    nc.tensor.matmul(ps_nn[:n_w, :], ones_s[:1, :n_w],
                                     bnr_s[:], start=False, stop=True)
                o_sb = npool.tile([128, D], dt.float32, tag="o_sb")
                nc.scalar.activation(o_sb[:n_w, :], ps_nn[:n_w, :], relu)
                r_sb = npool.tile([128, D], cdt, tag="r_sb")
                nc.vector.tensor_add(r_sb[:n_w, :], o_sb[:n_w, :], xb[:n_w, :])
                # layernorm over free dim
                st6 = npool.tile([128, 6], dt.float32, tag="st6")
                nc.vector.bn_stats(st6[:n_w, :], r_sb[:n_w, :])
                mv = npool.tile([128, 2], dt.float32, tag="mv")
                nc.vector.bn_aggr(mv[:n_w, :], st6[:n_w, :])
                sd = npool.tile([128, 1], dt.float32, tag="sd")
                nc.scalar.activation(sd[:n_w, :], mv[:n_w, 1:2],
                                     mybir.ActivationFunctionType.Sqrt,
                                     bias=eps_t[:n_w, :])
                rstd = npool.tile([128, 1], dt.float32, tag="rstd")
                nc.vector.reciprocal(rstd[:n_w, :], sd[:n_w, :])
                y = npool.tile([128, D], cdt, tag="y")
                nc.vector.tensor_scalar(y[:n_w, :], r_sb[:n_w, :],
                                        mv[:n_w, 0:1], rstd[:n_w, :],
                                        op0=mybir.AluOpType.subtract,
                                        op1=mybir.AluOpType.mult)
                if not skip_affine:
                    y2 = npool.tile([128, D], dt.float32, tag="y2")
                    nc.vector.tensor_mul(y2[:n_w, :], y[:n_w, :],
                                         gma_s[:n_w, :])
                    y3 = npool.tile([128, D], cdt, tag="y3")
                    nc.vector.tensor_add(y3[:n_w, :], y2[:n_w, :],
                                         bta_s[:n_w, :])
                    y = y3
                nc.sync.dma_start(out=out[b * BLK:b * BLK + n_w, :],
                                  in_=y[:n_w, :])

            # ---------------- edge phase (2-super pipeline skew) -----------
            gbufs = {}

            def issue_batch(bi):
                if bi >= n_batches or bi in gbufs:
                    return
                e0 = bi * GBATCH
                g_n = min(GBATCH, L_tot - e0)
                w = g_n // 2
                gt = gpool.tile([128, GBATCH], cdt, tag="gbuf")
                nc.sync.dma_start(out=gt[:, :g_n], in_=gx_t[:, e0:e0 + g_n])
                et = eapool.tile([2 * EA, GBATCH // 2], cdt, tag="eab")
                nc.sync.dma_start(out=et[:, :w],
                                  in_=ea2[:, e0 // 2:e0 // 2 + w])
                ot = ohpool.tile([CHUNK, GBATCH], dt.float8e4, tag="ohb")
                nc.sync.dma_start(out=ot[:, :g_n], in_=oh_t[:, e0:e0 + g_n])
                gbufs[bi] = (gt, et, ot, w)

            issue_batch(0)
            issue_batch(1)
            # heavier consts load behind the first edge batches; none are
            # needed until the first scatter/node phase
            wna_s = load_const(wna[:], [D, D])
            wnb_s = load_const(wnb[:], [D, D])
            b1_s = load_const(b1[:], [D, 1], dt.float32)
            xt_s = load_const(xt_loc[:], [D, NPC])

            h_q = {}
            eh_q = {}
            ps2_q = {}
            xb_q = {}
            state = dict(an=None, b=None)
            blk_start_of = np.repeat(blk_start, n_chunks)

            def do_mm2(s2, kk):
                (h, off), ps2 = h_q[s2], ps2_q[s2]
                ksl = slice(kk * CHUNK, (kk + 1) * CHUNK)
                hsl = slice(off + kk * CHUNK, off + (kk + 1) * CHUNK)
                nc.tensor.matmul(ps2[:, ksl], h[:, hsl], w2_s[:],
                                 start=True, stop=skip_bias2)
                if not skip_bias2:
                    nc.tensor.matmul(ps2[:, ksl], ones_s[:], b2r_s[:],
                                     start=False, stop=True)

            def flush_node(t_now):
                if state.get("pend") and (t_now is None
                                          or t_now >= state["pend"][0] + 3):
                    _, b, an_ps, agg_sb, xb = state.pop("pend")
                    node_mlp(b, an_ps, agg_sb, xb)

            def do_scatter(s3, kk):
                eh = eh_q[s3]
                t = (SUPER // CHUNK) * s3 + kk
                flush_node(t)
                b = int(blk_of_chunk[t])
                nb_c = int(n_chunks[b])
                rel = t - int(blk_start_of[t])
                # two alternating accumulators (A/B halves of one packed PSUM
                # tile) break the PSUM-RAW accumulate chain; third slice is
                # the node-MLP accumulator for this block
                if rel == 0:
                    an_ps = psum.tile([128, BLK + D], dt.float32, tag="an_ps")
                    state["an"] = an_ps
                    n_w = min(BLK, NPC - b * BLK)
                    xb = npool.tile([128, D], dt.float32, tag="xb")
                    nc.sync.dma_start(out=xb[:n_w, :],
                                      in_=x_loc[b * BLK:b * BLK + n_w, :])
                    xb_q[b] = xb
                if rel == 1:
                    b_ps = psum.tile([128, BLK], dt.float32, tag="b_ps")
                    state["b"] = b_ps
                p = rel % 2
                acc = state["an"][:, 0:BLK] if p == 0 else state["b"][:]
                last_rel = nb_c - 1 - ((nb_c - 1 - p) % 2)
                bo, to = divmod(t, GBATCH // CHUNK)
                ot = gbufs[bo][2]
                ksl = slice(kk * CHUNK, (kk + 1) * CHUNK)
                nc.tensor.matmul(acc, eh[:, ksl],
                                 ot[:, to * CHUNK:(to + 1) * CHUNK],
                                 start=rel < 2, stop=rel == last_rel)
                if rel == nb_c - 1:
                    flush_node(None)   # never hold two pending blocks
                    agg_sb = npool.tile([128, BLK], cdt, tag="agg")
                    if nb_c >= 2:
                        # tensor ops may read only one PSUM operand: stage B
                        bsb = npool.tile([128, BLK], cdt, tag="bsb")
                        nc.vector.tensor_copy(bsb[:], state["b"][:])
                        nc.vector.tensor_add(agg_sb[:], state["an"][:, 0:BLK],
                                             bsb[:])
                    else:
                        nc.vector.tensor_copy(agg_sb[:], state["an"][:, 0:BLK])
                    # defer the node matmuls a few chunk slots so the PE
                    # never waits on the DVE merge
                    state["pend"] = (t, b, state["an"], agg_sb, xb_q.pop(b))

            def mm1a(sx, ps1):
                bi, k = divmod(sx, GBATCH // SUPER)
                et, w = gbufs[bi][1], gbufs[bi][3]
                a, c0 = divmod(k * SUPER, w)
                nc.tensor.matmul(ps1[:],
                                 w1b_s[32 * a:32 * a + 32, :],
                                 et[32 * a:32 * a + 32, c0:c0 + SUPER],
                                 start=True, stop=False)

            def mm1b(sx, ps1):
                bi, k = divmod(sx, GBATCH // SUPER)
                gt = gbufs[bi][0]
                nc.tensor.matmul(ps1[:], w1a_s[:],
                                 gt[:, k * SUPER:(k + 1) * SUPER],
                                 start=False, stop=True)

            for it in range(n_sup + 2):
                s1, s2, s3 = it, it - 1, it - 2
                # mm1 runs on PAIRS of supers (even iterations) so the w1a/w1b
                # stationaries are reused and the ps1 accumulate chains of the
                # two supers hide each other's latency
                em1 = s1 < n_sup and s1 % 2 == 0
                pw = 2 if s1 + 1 < n_sup else 1
                em2 = 0 <= s2 < n_sup
                em3 = s3 >= 0
                if em1:
                    bi, k = divmod(s1, GBATCH // SUPER)
                    if k == 0:
                        issue_batch(bi + 1)
                    ps1_l = []
                    for j in range(pw):
                        ps1j = psum.tile([128, SUPER], dt.float32, tag="ps1")
                        ps1_l.append(ps1j)
                        mm1a(s1 + j, ps1j)
                if em2:
                    ps2 = psum.tile([128, SUPER], dt.float32, tag="ps2")
                    ps2_q[s2] = ps2
                    do_mm2(s2, 0)
                if em3:
                    do_scatter(s3, 0)
                if em2:
                    do_mm2(s2, 1)
                if em3:
                    do_scatter(s3, 1)
                if em1:
                    for j in range(pw):
                        mm1b(s1 + j, ps1_l[j])
                    for j in range(pw):
                        h = hpool.tile([128, SUPER], cdt, tag="h")
                        nc.scalar.activation(h[:], ps1_l[j][:], relu,
                                             bias=b1_s[:])
                        h_q[s1 + j] = (h, 0)
                if em2:
                    do_mm2(s2, 2)
                if em3:
                    do_scatter(s3, 2)
                if em2:
                    do_mm2(s2, 3)
                if em3:
                    do_scatter(s3, 3)
                    eh_q.pop(s3)
                if em2:
                    h_q.pop(s2)
                    eh = epool.tile([128, SUPER], cdt, tag="eh")
                    if s2 % 8 == 0:
                        nc.scalar.activation(eh[:], ps2_q[s2][:], relu)
                    else:
                        nc.vector.tensor_scalar_max(eh[:], ps2_q[s2][:], 0.0)
                    ps2_q.pop(s2)
                    eh_q[s2] = eh
            flush_node(None)

    nc.compile()
    return nc


def _prepare(**inputs):
    x = np.ascontiguousarray(np.asarray(inputs["x"], F32))
    ei = np.asarray(inputs["edge_index"]).astype(np.int64)
    ea = np.ascontiguousarray(np.asarray(inputs["edge_attr"], F32))
    W_e1 = np.asarray(inputs["W_e1"], F32)
    b_e1 = np.asarray(inputs["b_e1"], F32)
    W_e2 = np.asarray(inputs["W_e2"], F32)
    b_e2 = np.asarray(inputs["b_e2"], F32)
    W_n = np.asarray(inputs["W_n"], F32)
    b_n = np.asarray(inputs["b_n"], F32)
    gamma = np.asarray(inputs["gamma"], F32)
    beta = np.asarray(inputs["beta"], F32)

    cnp = _np_cdt()
    dest, src = ei[0], ei[1]
    sched, per_core = _build_schedule(dest, src)
    sched["skip_bias2"] = bool(np.all(b_e2 == 0))
    sched["skip_biasn"] = bool(np.all(b_n == 0))
    sched["skip_affine"] = bool(np.all(gamma == 1) and np.all(beta == 0))
    nc = _build_bass(sched)

    import ml_dtypes
    L_tot = sched["L_tot"]
    T_tot = sched["T_tot"]
    ones_r = np.ones((1, 128), cnp)
    iota_d = np.arange(BLK, dtype=F32)
    gma = np.tile(gamma[None, :], (128, 1)).astype(F32)
    bta = np.tile(beta[None, :], (128, 1)).astype(F32)

    ea_z = np.concatenate([ea, np.zeros((1, EA), F32)], axis=0)  # -1 -> zeros
    x_cdt = x.astype(cnp)

    def pack_ea(ea_pad):
        """[L, 32] -> [64, L/2]: row 32a+d, col c = ea_pad[a*w + c, d] per
        GBATCH-edge batch of width w = g_n//2."""
        outs = []
        for e0 in range(0, L_tot, GBATCH):
            g_n = min(GBATCH, L_tot - e0)
            w = g_n // 2
            blk = ea_pad[e0:e0 + g_n].reshape(2, w, EA)
            outs.append(blk.transpose(0, 2, 1).reshape(2 * EA, w))
        return np.ascontiguousarray(np.concatenate(outs, axis=1))

    in_maps = []
    for c in range(NC):
        pc = per_core[c]
        gx = np.ascontiguousarray(x_cdt[pc["src"]].T)       # [128, L] bf16
        ea2 = pack_ea(ea_z[pc["ea_perm"]].astype(cnp))      # [64, L/2]
        dr = pc["dest_rel"].reshape(T_tot, CHUNK)           # [T, e]
        oh = (dr[:, :, None] == iota_d[None, None, :])      # [T, e, d]
        oh_t = np.ascontiguousarray(
            oh.transpose(1, 0, 2).reshape(CHUNK, L_tot)
        ).astype(ml_dtypes.float8_e4m3fn)
        xs = x[c * NPC:(c + 1) * NPC]
        in_maps.append({
            "gx_t": gx, "ea2": ea2, "oh_t": oh_t,
            "xt_loc": np.ascontiguousarray(xs.T.astype(cnp)),
            "x_loc": xs,
            "w1a": np.ascontiguousarray(W_e1[:D].astype(cnp)),
            "w1b2": np.ascontiguousarray(np.tile(W_e1[D:], (2, 1)).astype(cnp)),
            "w2": W_e2.astype(cnp),
            "wna": np.ascontiguousarray(W_n[:D].astype(cnp)),
            "wnb": np.ascontiguousarray(W_n[D:].astype(cnp)),
            "b1": b_e1[:, None].copy(),
            "b2r": b_e2[None, :].astype(cnp),
            "bnr": b_n[None, :].astype(cnp),
            "ones_r": ones_r, "gma": gma, "bta": bta,
        })
    return nc, in_maps


def kernel(**inputs) -> np.ndarray:
    nc, in_maps = _prepare(**inputs)
    from concourse.bass_utils import run_bass_kernel_spmd
    res = run_bass_kernel_spmd(nc, in_maps, list(range(NC)))
    return np.concatenate(
        [np.asarray(res.results[c]["out"]) for c in range(NC)],
        axis=0).astype(np.float32)


# revision 20
# speedup vs baseline: 1.0110x; 1.0110x over previous
"""DMPNN layer on 8 Trainium2 NeuronCores.

Sharding: edges are assigned to the core that owns their *destination* node
(50000 nodes / 8 cores = 6250 each), so the scatter-sum is core-local and no
collectives are needed.  The per-edge source-feature gather is done on the
HOST (same as the edge_attr permutation): each core receives a dense
feature-major stream gx_t = x[src].T for its (padded) edge list, so the
device never runs gpsimd dma_gather — the old bottleneck (~10ns/edge of Q7
descriptor generation, 905us total).

Within a core, edges are grouped by 128-node destination block (scatter-sum
is an accumulating onehot-matmul into one PSUM tile per block).  Per-block
chunk counts are maxed across cores so all 8 cores run the same static
program (SPMD); per-core variation is data only (gx/ea/dest_rel, padded with
dummy edges whose dest_rel=-1 masks them out of the scatter).

Datapath is bf16 (fp32 PSUM accumulation).  The main loop is software-
pipelined with a 2-super skew (mm1 of super s, mm2 of s-1, scatter of s-2)
so the PE never waits on the scalar-engine relus.
"""

import os

# The bass kernel executes through jax's axon/neuron platform.  A stray
# JAX_PLATFORMS=cpu (commonly set to keep jax off neuronxcc) would hide the
# NeuronCores, so drop it before jax is first imported.
if os.environ.get("JAX_PLATFORMS", "").strip() == "cpu":
    os.environ.pop("JAX_PLATFORMS")

import numpy as np

N_NODES = 50000
N_EDGES = 640000
D = 128          # node feature dim == hidden == output dim
EA = 32          # edge attr dim
NC = 8           # cores
NPC = N_NODES // NC   # nodes per core
BLK = 128        # node block width (scatter psum tile)
NB = (NPC + BLK - 1) // BLK   # 49 blocks per core (last one 106 nodes)
CHUNK = 128      # edge chunk (scatter/matmul granularity)
SUPER = 512      # edge super-chunk (mm1/relu batching)
GBATCH = 4096    # edges per DMA batch (1MB bf16 -> near-peak HBM bw)
EPS = 1e-5

F32 = np.float32


def _np_cdt():
    import ml_dtypes
    return ml_dtypes.bfloat16


def _build_schedule(dest: np.ndarray, src: np.ndarray):
    """Group edges by (core, block); pad so the chunk structure is identical
    across cores.  Returns shared schedule + per-core data."""
    core = dest // NPC
    block = (dest % NPC) // BLK

    key = core * NB + block
    order = np.argsort(key, kind="stable")
    key_s = key[order]
    cnt = np.bincount(key, minlength=NC * NB).reshape(NC, NB)

    # shared chunk counts per block: max over cores, >= 1, total a SUPER mult
    n_chunks = np.maximum(1, -(-cnt.max(axis=0) // CHUNK))  # [NB]
    extra = (-int(n_chunks.sum())) % (SUPER // CHUNK)
    n_chunks[NB - 1] += extra
    T_tot = int(n_chunks.sum())
    L_tot = T_tot * CHUNK

    # padded start offset of each block group within a core's stream
    pad_start = np.concatenate([[0], np.cumsum(n_chunks)[:-1]]) * CHUNK

    # rank of each edge within its (core, block) group
    grp_start = np.zeros(NC * NB + 1, np.int64)
    np.cumsum(np.bincount(key, minlength=NC * NB), out=grp_start[1:])
    rank = np.arange(N_EDGES) - grp_start[key_s]

    b_s = key_s % NB
    c_s = key_s // NB
    pos = pad_start[b_s] + rank

    blk_of_chunk = np.repeat(np.arange(NB), n_chunks)
    blk_of_edge = np.repeat(blk_of_chunk, CHUNK)

    per_core = []
    dest_s = dest[order]
    src_s = src[order]
    for c in range(NC):
        m = c_s == c
        p = pos[m]
        src_pad = np.zeros(L_tot, np.int64)
        src_pad[p] = src_s[m]
        dest_rel = np.full(L_tot, -1.0, F32)
        dest_rel[p] = (dest_s[m] % NPC - blk_of_edge[p] * BLK).astype(F32)
        assert dest_rel.max() < BLK and (dest_rel[p] >= 0).all()
        ea_perm = np.full(L_tot, -1, np.int64)
        ea_perm[p] = order[m]   # original edge id per padded slot (-1 = dummy)
        per_core.append(dict(src=src_pad, dest_rel=dest_rel, ea_perm=ea_perm))

    sched = dict(n_chunks=n_chunks, T_tot=T_tot, L_tot=L_tot,
                 blk_of_chunk=blk_of_chunk)
    return sched, per_core


def _build_bass(sched):
    import concourse.bacc as bacc
    import concourse.mybir as mybir
    import concourse.tile as tile

    dt = mybir.dt
    cdt = dt.bfloat16
    T_tot = sched["T_tot"]
    L_tot = sched["L_tot"]
    n_chunks = sched["n_chunks"]
    blk_of_chunk = sched["blk_of_chunk"]
    skip_bias2 = sched["skip_bias2"]
    skip_biasn = sched["skip_biasn"]
    skip_affine = sched["skip_affine"]

    n_sup = T_tot // (SUPER // CHUNK)
    n_batches = -(-L_tot // GBATCH)

    # first/last chunk of each block
    blk_start = np.concatenate([[0], np.cumsum(n_chunks)[:-1]])
    first_of = np.zeros(T_tot, bool)
    last_of = np.zeros(T_tot, bool)
    first_of[blk_start] = True
    last_of[blk_start + n_chunks - 1] = True

    nc = bacc.Bacc("TRN2", target_bir_lowering=False, debug=False,
                   num_devices=NC)

    def din(name, shape, d=None):
        return nc.dram_tensor(name, shape, d or cdt, kind="ExternalInput").ap()

    gx_t = din("gx_t", [D, L_tot])
    ea2 = din("ea2", [4 * EA, L_tot // 2])
    oh_t = din("oh_t", [CHUNK, L_tot], dt.float8e4)  # host-built onehots
    xt_loc = din("xt_loc", [D, NPC])
    x_loc = din("x_loc", [NPC, D], dt.float32)
    w1a = din("w1a", [D, D])
    w1b2 = din("w1b2", [4 * EA, D])   # w1b/2 dup-packed at partitions 0/64
    w2 = din("w2", [D, D])
    wna = din("wna", [D, D])
    wnb = din("wnb", [D, D])
    b1 = din("b1", [D, 1], dt.float32)
    b2r = din("b2r", [1, D])
    bnr = din("bnr", [1, D])
    ones_r = din("ones_r", [1, 128])
    gma = din("gma", [128, D], dt.float32)
    bta = din("bta", [128, D], dt.float32)
    out = nc.dram_tensor("out", [NPC, D], cdt, kind="ExternalOutput").ap()

    with tile.TileContext(nc) as tc:
        from contextlib import ExitStack
        ctx = ExitStack()
        with ctx:
            const = ctx.enter_context(tc.tile_pool(name="const", bufs=1))
            gpool = ctx.enter_context(tc.tile_pool(name="gx", bufs=2))
            eapool = ctx.enter_context(tc.tile_pool(name="ea", bufs=2))
            hpool = ctx.enter_context(tc.tile_pool(name="h", bufs=3))
            epool = ctx.enter_context(tc.tile_pool(name="eh", bufs=3))
            ohpool = ctx.enter_context(tc.tile_pool(name="ohp", bufs=3))
            psum = ctx.enter_context(tc.tile_pool(name="psum", bufs=2,
                                                  space="PSUM"))
            npool = ctx.enter_context(tc.tile_pool(name="node", bufs=3))

            def load_const(ap, shape, d=None):
                t = const.tile(shape, d or cdt, tag=f"c_{ap.tensor.name}")
                nc.sync.dma_start(out=t[:], in_=ap)
                return t

            w1a_s = load_const(w1a[:], [D, D])
            w1b_s = load_const(w1b2[:], [4 * EA, D])
            w2_s = load_const(w2[:], [D, D])
            if not skip_bias2:
                b2r_s = load_const(b2r[:], [1, D])
            if not (skip_bias2 and skip_biasn):
                ones_s = load_const(ones_r[:], [1, 128])
            if not skip_biasn:
                bnr_s = load_const(bnr[:], [1, D])
            if not skip_affine:
                gma_s = load_const(gma[:], [128, D], dt.float32)
                bta_s = load_const(bta[:], [128, D], dt.float32)

            eps_t = const.tile([128, 1], dt.float32, tag="eps")
            nc.vector.memset(eps_t[:], EPS)

            relu = mybir.ActivationFunctionType.Relu

            def node_mlp(b, an_ps, agg_sb, xb):
                """node MLP + residual layernorm for block b, consuming its
                merged scatter accumulator (SBUF bf16)."""
                n_w = min(BLK, NPC - b * BLK)
                cols = slice(b * BLK, b * BLK + n_w)
                ps_nn = an_ps[:, BLK:BLK + D]
                nc.tensor.matmul(ps_nn[:n_w, :], xt_s[:, cols], wna_s[:],
                                 start=True, stop=False)
                nc.tensor.matmul(ps_nn[:n_w, :], agg_sb[:, :n_w], wnb_s[:],
                                 start=False, stop=skip_biasn)
                if not skip_biasn:
                    nc.tensor.matmul(ps_nn[:n_w, :], ones_s[:1, :n_w],
                                     bnr_s[:], start=False, stop=True)
                o_sb = npool.tile([128, D], dt.float32, tag="o_sb")
                nc.scalar.activation(o_sb[:n_w, :], ps_nn[:n_w, :], relu)
                r_sb = npool.tile([128, D], cdt, tag="r_sb")
                nc.vector.tensor_add(r_sb[:n_w, :], o_sb[:n_w, :], xb[:n_w, :])
                # layernorm over free dim
                st6 = npool.tile([128, 6], dt.float32, tag="st6")
                nc.vector.bn_stats(st6[:n_w, :], r_sb[:n_w, :])
                mv = npool.tile([128, 2], dt.float32, tag="mv")
                nc.vector.bn_aggr(mv[:n_w, :], st6[:n_w, :])
                sd = npool.tile([128, 1], dt.float32, tag="sd")
                nc.scalar.activation(sd[:n_w, :], mv[:n_w, 1:2],
                                     mybir.ActivationFunctionType.Sqrt,
                                     bias=eps_t[:n_w, :])
                rstd = npool.tile([128, 1], dt.float32, tag="rstd")
                nc.vector.reciprocal(rstd[:n_w, :], sd[:n_w, :])
                y = npool.tile([128, D], cdt, tag="y")
                nc.vector.tensor_scalar(y[:n_w, :], r_sb[:n_w, :],
                                        mv[:n_w, 0:1], rstd[:n_w, :],
                                        op0=mybir.AluOpType.subtract,
                                        op1=mybir.AluOpType.mult)
                if not skip_affine:
                    y2 = npool.tile([128, D], dt.float32, tag="y2")
                    nc.vector.tensor_mul(y2[:n_w, :], y[:n_w, :],
                                         gma_s[:n_w, :])
                    y3 = npool.tile([128, D], cdt, tag="y3")
                    nc.vector.tensor_add(y3[:n_w, :], y2[:n_w, :],
                                         bta_s[:n_w, :])
                    y = y3
                nc.sync.dma_start(out=out[b * BLK:b * BLK + n_w, :],
                                  in_=y[:n_w, :])

            # ---------------- edge phase (2-super pipeline skew) -----------
            gbufs = {}

            def issue_batch(bi):
                if bi >= n_batches or bi in gbufs:
                    return
                e0 = bi * GBATCH
                g_n = min(GBATCH, L_tot - e0)
                w = g_n // 2
                gt = gpool.tile([128, GBATCH], cdt, tag="gbuf")
                nc.sync.dma_start(out=gt[:, :g_n], in_=gx_t[:, e0:e0 + g_n])
                et = eapool.tile([4 * EA, GBATCH // 2], cdt, tag="eab")
                nc.sync.dma_start(out=et[:, :w],
                                  in_=ea2[:, e0 // 2:e0 // 2 + w])
                ot = ohpool.tile([CHUNK, GBATCH], dt.float8e4, tag="ohb")
                nc.sync.dma_start(out=ot[:, :g_n], in_=oh_t[:, e0:e0 + g_n])
                gbufs[bi] = (gt, et, ot, w)

            issue_batch(0)
            issue_batch(1)
            # heavier consts load behind the first edge batches; none are
            # needed until the first scatter/node phase
            wna_s = load_const(wna[:], [D, D])
            wnb_s = load_const(wnb[:], [D, D])
            b1_s = load_const(b1[:], [D, 1], dt.float32)
            xt_s = load_const(xt_loc[:], [D, NPC])

            h_q = {}
            eh_q = {}
            ps2_q = {}
            xb_q = {}
            state = dict(an=None, b=None)
            blk_start_of = np.repeat(blk_start, n_chunks)

            def do_mm2(s2, kk):
                (h, off), ps2 = h_q[s2], ps2_q[s2]
                ksl = slice(kk * CHUNK, (kk + 1) * CHUNK)
                hsl = slice(off + kk * CHUNK, off + (kk + 1) * CHUNK)
                nc.tensor.matmul(ps2[:, ksl], h[:, hsl], w2_s[:],
                                 start=True, stop=skip_bias2)
                if not skip_bias2:
                    nc.tensor.matmul(ps2[:, ksl], ones_s[:], b2r_s[:],
                                     start=False, stop=True)

            def flush_node(t_now):
                if state.get("pend") and (t_now is None
                                          or t_now >= state["pend"][0] + 3):
                    _, b, an_ps, agg_sb, xb = state.pop("pend")
                    node_mlp(b, an_ps, agg_sb, xb)

            def do_scatter(s3, kk):
                eh = eh_q[s3]
                t = (SUPER // CHUNK) * s3 + kk
                flush_node(t)
                b = int(blk_of_chunk[t])
                nb_c = int(n_chunks[b])
                rel = t - int(blk_start_of[t])
                # two alternating accumulators (A/B halves of one packed PSUM
                # tile) break the PSUM-RAW accumulate chain; third slice is
                # the node-MLP accumulator for this block
                if rel == 0:
                    an_ps = psum.tile([128, BLK + D], dt.float32, tag="an_ps")
                    state["an"] = an_ps
                    n_w = min(BLK, NPC - b * BLK)
                    xb = npool.tile([128, D], dt.float32, tag="xb")
                    nc.sync.dma_start(out=xb[:n_w, :],
                                      in_=x_loc[b * BLK:b * BLK + n_w, :])
                    xb_q[b] = xb
                if rel == 1:
                    b_ps = psum.tile([128, BLK], dt.float32, tag="b_ps")
                    state["b"] = b_ps
                p = rel % 2
                acc = state["an"][:, 0:BLK] if p == 0 else state["b"][:]
                last_rel = nb_c - 1 - ((nb_c - 1 - p) % 2)
                bo, to = divmod(t, GBATCH // CHUNK)
                ot = gbufs[bo][2]
                ksl = slice(kk * CHUNK, (kk + 1) * CHUNK)
                nc.tensor.matmul(acc, eh[:, ksl],
                                 ot[:, to * CHUNK:(to + 1) * CHUNK],
                                 start=rel < 2, stop=rel == last_rel)
                if rel == nb_c - 1:
                    flush_node(None)   # never hold two pending blocks
                    agg_sb = npool.tile([128, BLK], cdt, tag="agg")
                    if nb_c >= 2:
                        # tensor ops may read only one PSUM operand: stage B
                        bsb = npool.tile([128, BLK], cdt, tag="bsb")
                        nc.vector.tensor_copy(bsb[:], state["b"][:])
                        nc.vector.tensor_add(agg_sb[:], state["an"][:, 0:BLK],
                                             bsb[:])
                    else:
                        nc.vector.tensor_copy(agg_sb[:], state["an"][:, 0:BLK])
                    # defer the node matmuls a few chunk slots so the PE
                    # never waits on the DVE merge
                    state["pend"] = (t, b, state["an"], agg_sb, xb_q.pop(b))

            def mm1a(sx, ps1):
                bi, k = divmod(sx, GBATCH // SUPER)
                gt = gbufs[bi][0]
                nc.tensor.matmul(ps1[:], w1a_s[:],
                                 gt[:, k * SUPER:(k + 1) * SUPER],
                                 start=True, stop=False)

            def mm1b(sx, ps1):
                bi, k = divmod(sx, GBATCH // SUPER)
                et, w = gbufs[bi][1], gbufs[bi][3]
                a, c0 = divmod(k * SUPER, w)
                nc.tensor.matmul(ps1[:],
                                 w1b_s[64 * a:64 * a + 64, :],
                                 et[64 * a:64 * a + 64, c0:c0 + SUPER],
                                 start=False, stop=True)

            for it in range(n_sup + 2):
                s1, s2, s3 = it, it - 1, it - 2
                # mm1 runs on PAIRS of supers (even iterations) so the w1a/w1b
                # stationaries are reused and the ps1 accumulate chains of the
                # two supers hide each other's latency
                em1 = s1 < n_sup and s1 % 2 == 0
                pw = 2 if s1 + 1 < n_sup else 1
                em2 = 0 <= s2 < n_sup
                em3 = s3 >= 0
                if em1:
                    bi, k = divmod(s1, GBATCH // SUPER)
                    if k == 0:
                        issue_batch(bi + 1)
                    ps1_l = []
                    for j in range(pw):
                        ps1j = psum.tile([128, SUPER], dt.float32, tag="ps1")
                        ps1_l.append(ps1j)
                        mm1a(s1 + j, ps1j)
                if em2:
                    ps2 = psum.tile([128, SUPER], dt.float32, tag="ps2")
                    ps2_q[s2] = ps2
                    do_mm2(s2, 0)
                if em3:
                    do_scatter(s3, 0)
                if em2:
                    do_mm2(s2, 1)
                if em3:
                    do_scatter(s3, 1)
                if em1:
                    for j in range(pw):
                        mm1b(s1 + j, ps1_l[j])
                    for j in range(pw):
                        h = hpool.tile([128, SUPER], cdt, tag="h")
                        nc.scalar.activation(h[:], ps1_l[j][:], relu,
                                             bias=b1_s[:])
                        h_q[s1 + j] = (h, 0)
                if em2:
                    do_mm2(s2, 2)
                if em3:
                    do_scatter(s3, 2)
                if em2:
                    do_mm2(s2, 3)
                if em3:
                    do_scatter(s3, 3)
                    eh_q.pop(s3)
                if em2:
                    h_q.pop(s2)
                    eh = epool.tile([128, SUPER], cdt, tag="eh")
                    if s2 % 8 == 0:
                        nc.scalar.activation(eh[:], ps2_q[s2][:], relu)
                    else:
                        nc.vector.tensor_scalar_max(eh[:], ps2_q[s2][:], 0.0)
                    ps2_q.pop(s2)
                    eh_q[s2] = eh
            flush_node(None)

    nc.compile()
    return nc


def _prepare(**inputs):
    x = np.ascontiguousarray(np.asarray(inputs["x"], F32))
    ei = np.asarray(inputs["edge_index"]).astype(np.int64)
    ea = np.ascontiguousarray(np.asarray(inputs["edge_attr"], F32))
    W_e1 = np.asarray(inputs["W_e1"], F32)
    b_e1 = np.asarray(inputs["b_e1"], F32)
    W_e2 = np.asarray(inputs["W_e2"], F32)
    b_e2 = np.asarray(inputs["b_e2"], F32)
    W_n = np.asarray(inputs["W_n"], F32)
    b_n = np.asarray(inputs["b_n"], F32)
    gamma = np.asarray(inputs["gamma"], F32)
    beta = np.asarray(inputs["beta"], F32)

    cnp = _np_cdt()
    dest, src = ei[0], ei[1]
    sched, per_core = _build_schedule(dest, src)
    sched["skip_bias2"] = bool(np.all(b_e2 == 0))
    sched["skip_biasn"] = bool(np.all(b_n == 0))
    sched["skip_affine"] = bool(np.all(gamma == 1) and np.all(beta == 0))
    nc = _build_bass(sched)

    import ml_dtypes
    L_tot = sched["L_tot"]
    T_tot = sched["T_tot"]
    ones_r = np.ones((1, 128), cnp)
    iota_d = np.arange(BLK, dtype=F32)
    gma = np.tile(gamma[None, :], (128, 1)).astype(F32)
    bta = np.tile(beta[None, :], (128, 1)).astype(F32)

    ea_z = np.concatenate([ea, np.zeros((1, EA), F32)], axis=0)  # -1 -> zeros
    x_cdt = x.astype(cnp)

    def pack_ea(ea_pad):
        """[L, 32] -> [128, L/2]: rows 64a+d (d<64, feature d%32 duplicated
        so the matmul contracts K=64), col c = edge a*w + c per GBATCH batch
        of width w = g_n//2.  The duplicate is cancelled by w1b/2."""
        outs = []
        for e0 in range(0, L_tot, GBATCH):
            g_n = min(GBATCH, L_tot - e0)
            w = g_n // 2
            blk = ea_pad[e0:e0 + g_n].reshape(2, w, EA)
            blk = np.concatenate([blk, blk], axis=2)          # [2, w, 64]
            outs.append(blk.transpose(0, 2, 1).reshape(4 * EA, w))
        return np.ascontiguousarray(np.concatenate(outs, axis=1))

    in_maps = []
    for c in range(NC):
        pc = per_core[c]
        gx = np.ascontiguousarray(x_cdt[pc["src"]].T)       # [128, L] bf16
        ea2 = pack_ea(ea_z[pc["ea_perm"]].astype(cnp))      # [64, L/2]
        dr = pc["dest_rel"].reshape(T_tot, CHUNK)           # [T, e]
        oh = (dr[:, :, None] == iota_d[None, None, :])      # [T, e, d]
        oh_t = np.ascontiguousarray(
            oh.transpose(1, 0, 2).reshape(CHUNK, L_tot)
        ).astype(ml_dtypes.float8_e4m3fn)
        xs = x[c * NPC:(c + 1) * NPC]
        in_maps.append({
            "gx_t": gx, "ea2": ea2, "oh_t": oh_t,
            "xt_loc": np.ascontiguousarray(xs.T.astype(cnp)),
            "x_loc": xs,
            "w1a": np.ascontiguousarray(W_e1[:D].astype(cnp)),
            "w1b2": np.ascontiguousarray(
                np.tile(W_e1[D:] * 0.5, (4, 1)).astype(cnp)),
            "w2": W_e2.astype(cnp),
            "wna": np.ascontiguousarray(W_n[:D].astype(cnp)),
            "wnb": np.ascontiguousarray(W_n[D:].astype(cnp)),
            "b1": b_e1[:, None].copy(),
            "b2r": b_e2[None, :].astype(cnp),
            "bnr": b_n[None, :].astype(cnp),
            "ones_r": ones_r, "gma": gma, "bta": bta,
        })
    return nc, in_maps


def kernel(**inputs) -> np.ndarray:
    nc, in_maps = _prepare(**inputs)
    from concourse.bass_utils import run_bass_kernel_spmd
    res = run_bass_kernel_spmd(nc, in_maps, list(range(NC)))
    return np.concatenate(
        [np.asarray(res.results[c]["out"]) for c in range(NC)],
        axis=0).astype(np.float32)


# revision 21
# speedup vs baseline: 1.0558x; 1.0444x over previous
"""DMPNN layer on 8 Trainium2 NeuronCores.

Sharding: edges are assigned to the core that owns their *destination* node
(50000 nodes / 8 cores = 6250 each), so the scatter-sum is core-local and no
collectives are needed.  The per-edge source-feature gather is done on the
HOST (same as the edge_attr permutation): each core receives a dense
feature-major stream gx_t = x[src].T for its (padded) edge list, so the
device never runs gpsimd dma_gather — the old bottleneck (~10ns/edge of Q7
descriptor generation, 905us total).

Within a core, edges are grouped by 128-node destination block (scatter-sum
is an accumulating onehot-matmul into one PSUM tile per block).  Per-block
chunk counts are maxed across cores so all 8 cores run the same static
program (SPMD); per-core variation is data only (gx/ea/dest_rel, padded with
dummy edges whose dest_rel=-1 masks them out of the scatter).

Datapath is bf16 (fp32 PSUM accumulation).  The main loop is software-
pipelined with a 2-super skew (mm1 of super s, mm2 of s-1, scatter of s-2)
so the PE never waits on the scalar-engine relus.
"""

import os

# The bass kernel executes through jax's axon/neuron platform.  A stray
# JAX_PLATFORMS=cpu (commonly set to keep jax off neuronxcc) would hide the
# NeuronCores, so drop it before jax is first imported.
if os.environ.get("JAX_PLATFORMS", "").strip() == "cpu":
    os.environ.pop("JAX_PLATFORMS")

import numpy as np

N_NODES = 50000
N_EDGES = 640000
D = 128          # node feature dim == hidden == output dim
EA = 32          # edge attr dim
NC = 8           # cores
NPC = N_NODES // NC   # nodes per core
BLK = 128        # node block width (scatter psum tile)
NB = (NPC + BLK - 1) // BLK   # 49 blocks per core (last one 106 nodes)
CHUNK = 128      # edge chunk (scatter/matmul granularity)
SUPER = 512      # edge super-chunk (mm1/relu batching)
GBATCH = 4096    # edges per DMA batch (1MB bf16 -> near-peak HBM bw)
EPS = 1e-5

F32 = np.float32


def _np_cdt():
    import ml_dtypes
    return ml_dtypes.bfloat16


def _build_schedule(dest: np.ndarray, src: np.ndarray):
    """Group edges by (core, block); pad so the chunk structure is identical
    across cores.  Returns shared schedule + per-core data."""
    core = dest // NPC
    block = (dest % NPC) // BLK

    key = core * NB + block
    order = np.argsort(key, kind="stable")
    key_s = key[order]
    cnt = np.bincount(key, minlength=NC * NB).reshape(NC, NB)

    # shared chunk counts per block: max over cores, >= 1, total a SUPER mult
    n_chunks = np.maximum(1, -(-cnt.max(axis=0) // CHUNK))  # [NB]
    extra = (-int(n_chunks.sum())) % (SUPER // CHUNK)
    n_chunks[NB - 1] += extra
    T_tot = int(n_chunks.sum())
    L_tot = T_tot * CHUNK

    # padded start offset of each block group within a core's stream
    pad_start = np.concatenate([[0], np.cumsum(n_chunks)[:-1]]) * CHUNK

    # rank of each edge within its (core, block) group
    grp_start = np.zeros(NC * NB + 1, np.int64)
    np.cumsum(np.bincount(key, minlength=NC * NB), out=grp_start[1:])
    rank = np.arange(N_EDGES) - grp_start[key_s]

    b_s = key_s % NB
    c_s = key_s // NB
    pos = pad_start[b_s] + rank

    blk_of_chunk = np.repeat(np.arange(NB), n_chunks)
    blk_of_edge = np.repeat(blk_of_chunk, CHUNK)

    per_core = []
    dest_s = dest[order]
    src_s = src[order]
    for c in range(NC):
        m = c_s == c
        p = pos[m]
        src_pad = np.zeros(L_tot, np.int64)
        src_pad[p] = src_s[m]
        dest_rel = np.full(L_tot, -1.0, F32)
        dest_rel[p] = (dest_s[m] % NPC - blk_of_edge[p] * BLK).astype(F32)
        assert dest_rel.max() < BLK and (dest_rel[p] >= 0).all()
        ea_perm = np.full(L_tot, -1, np.int64)
        ea_perm[p] = order[m]   # original edge id per padded slot (-1 = dummy)
        per_core.append(dict(src=src_pad, dest_rel=dest_rel, ea_perm=ea_perm))

    sched = dict(n_chunks=n_chunks, T_tot=T_tot, L_tot=L_tot,
                 blk_of_chunk=blk_of_chunk)
    return sched, per_core


def _build_bass(sched):
    import concourse.bacc as bacc
    import concourse.mybir as mybir
    import concourse.tile as tile

    dt = mybir.dt
    cdt = dt.bfloat16
    T_tot = sched["T_tot"]
    L_tot = sched["L_tot"]
    n_chunks = sched["n_chunks"]
    blk_of_chunk = sched["blk_of_chunk"]
    skip_bias2 = sched["skip_bias2"]
    skip_biasn = sched["skip_biasn"]
    skip_affine = sched["skip_affine"]

    n_sup = T_tot // (SUPER // CHUNK)
    n_batches = -(-L_tot // GBATCH)

    # first/last chunk of each block
    blk_start = np.concatenate([[0], np.cumsum(n_chunks)[:-1]])
    first_of = np.zeros(T_tot, bool)
    last_of = np.zeros(T_tot, bool)
    first_of[blk_start] = True
    last_of[blk_start + n_chunks - 1] = True

    nc = bacc.Bacc("TRN2", target_bir_lowering=False, debug=False,
                   num_devices=NC)

    def din(name, shape, d=None):
        return nc.dram_tensor(name, shape, d or cdt, kind="ExternalInput").ap()

    gx_t = din("gx_t", [D, L_tot])
    ea2 = din("ea2", [2 * EA, L_tot // 2])
    oh_t = din("oh_t", [CHUNK, L_tot], dt.float8e4)  # host-built onehots
    xt_loc = din("xt_loc", [D, NPC])
    x_loc = din("x_loc", [NPC, D], dt.float32)
    w1a = din("w1a", [D, D])
    w1b2 = din("w1b2", [2 * EA, D])   # w1b replicated at partitions 0/32
    w2 = din("w2", [D, D])
    wna = din("wna", [D, D])
    wnb = din("wnb", [D, D])
    b1 = din("b1", [D, 1], dt.float32)
    b2r = din("b2r", [1, D])
    bnr = din("bnr", [1, D])
    ones_r = din("ones_r", [1, 128])
    gma = din("gma", [128, D], dt.float32)
    bta = din("bta", [128, D], dt.float32)
    out = nc.dram_tensor("out", [NPC, D], cdt, kind="ExternalOutput").ap()

    with tile.TileContext(nc) as tc:
        from contextlib import ExitStack
        ctx = ExitStack()
        with ctx:
            const = ctx.enter_context(tc.tile_pool(name="const", bufs=1))
            gpool = ctx.enter_context(tc.tile_pool(name="gx", bufs=3))
            eapool = ctx.enter_context(tc.tile_pool(name="ea", bufs=3))
            hpool = ctx.enter_context(tc.tile_pool(name="h", bufs=3))
            epool = ctx.enter_context(tc.tile_pool(name="eh", bufs=3))
            ohpool = ctx.enter_context(tc.tile_pool(name="ohp", bufs=4))
            psum = ctx.enter_context(tc.tile_pool(name="psum", bufs=2,
                                                  space="PSUM"))
            npool = ctx.enter_context(tc.tile_pool(name="node", bufs=3))

            def load_const(ap, shape, d=None):
                t = const.tile(shape, d or cdt, tag=f"c_{ap.tensor.name}")
                nc.sync.dma_start(out=t[:], in_=ap)
                return t

            w1a_s = load_const(w1a[:], [D, D])
            w1b_s = load_const(w1b2[:], [2 * EA, D])
            w2_s = load_const(w2[:], [D, D])
            if not skip_bias2:
                b2r_s = load_const(b2r[:], [1, D])
            if not (skip_bias2 and skip_biasn):
                ones_s = load_const(ones_r[:], [1, 128])
            if not skip_biasn:
                bnr_s = load_const(bnr[:], [1, D])
            if not skip_affine:
                gma_s = load_const(gma[:], [128, D], dt.float32)
                bta_s = load_const(bta[:], [128, D], dt.float32)

            eps_t = const.tile([128, 1], dt.float32, tag="eps")
            nc.vector.memset(eps_t[:], EPS)

            relu = mybir.ActivationFunctionType.Relu

            def node_mlp(b, an_ps, agg_sb, xb):
                """node MLP + residual layernorm for block b, consuming its
                merged scatter accumulator (SBUF bf16)."""
                n_w = min(BLK, NPC - b * BLK)
                cols = slice(b * BLK, b * BLK + n_w)
                ps_nn = an_ps[:, BLK:BLK + D]
                nc.tensor.matmul(ps_nn[:n_w, :], xt_s[:, cols], wna_s[:],
                                 start=True, stop=False)
                nc.tensor.matmul(ps_nn[:n_w, :], agg_sb[:, :n_w], wnb_s[:],
                                 start=False, stop=skip_biasn)
                if not skip_biasn:
                    nc.tensor.matmul(ps_nn[:n_w, :], ones_s[:1, :n_w],
                                     bnr_s[:], start=False, stop=True)
                o_sb = npool.tile([128, D], dt.float32, tag="o_sb")
                nc.scalar.activation(o_sb[:n_w, :], ps_nn[:n_w, :], relu)
                r_sb = npool.tile([128, D], cdt, tag="r_sb")
                nc.vector.tensor_add(r_sb[:n_w, :], o_sb[:n_w, :], xb[:n_w, :])
                # layernorm over free dim
                st6 = npool.tile([128, 6], dt.float32, tag="st6")
                nc.vector.bn_stats(st6[:n_w, :], r_sb[:n_w, :])
                mv = npool.tile([128, 2], dt.float32, tag="mv")
                nc.vector.bn_aggr(mv[:n_w, :], st6[:n_w, :])
                sd = npool.tile([128, 1], dt.float32, tag="sd")
                nc.scalar.activation(sd[:n_w, :], mv[:n_w, 1:2],
                                     mybir.ActivationFunctionType.Sqrt,
                                     bias=eps_t[:n_w, :])
                rstd = npool.tile([128, 1], dt.float32, tag="rstd")
                nc.vector.reciprocal(rstd[:n_w, :], sd[:n_w, :])
                y = npool.tile([128, D], cdt, tag="y")
                nc.vector.tensor_scalar(y[:n_w, :], r_sb[:n_w, :],
                                        mv[:n_w, 0:1], rstd[:n_w, :],
                                        op0=mybir.AluOpType.subtract,
                                        op1=mybir.AluOpType.mult)
                if not skip_affine:
                    y2 = npool.tile([128, D], dt.float32, tag="y2")
                    nc.vector.tensor_mul(y2[:n_w, :], y[:n_w, :],
                                         gma_s[:n_w, :])
                    y3 = npool.tile([128, D], cdt, tag="y3")
                    nc.vector.tensor_add(y3[:n_w, :], y2[:n_w, :],
                                         bta_s[:n_w, :])
                    y = y3
                nc.sync.dma_start(out=out[b * BLK:b * BLK + n_w, :],
                                  in_=y[:n_w, :])

            # ---------------- edge phase (2-super pipeline skew) -----------
            gbufs = {}

            def issue_batch(bi):
                if bi >= n_batches or bi in gbufs:
                    return
                e0 = bi * GBATCH
                g_n = min(GBATCH, L_tot - e0)
                w = g_n // 2
                gt = gpool.tile([128, GBATCH], cdt, tag="gbuf")
                nc.sync.dma_start(out=gt[:, :g_n], in_=gx_t[:, e0:e0 + g_n])
                et = eapool.tile([2 * EA, GBATCH // 2], cdt, tag="eab")
                nc.sync.dma_start(out=et[:, :w],
                                  in_=ea2[:, e0 // 2:e0 // 2 + w])
                ot = ohpool.tile([CHUNK, GBATCH], dt.float8e4, tag="ohb")
                nc.sync.dma_start(out=ot[:, :g_n], in_=oh_t[:, e0:e0 + g_n])
                gbufs[bi] = (gt, et, ot, w)

            issue_batch(0)
            issue_batch(1)
            # heavier consts load behind the first edge batches; none are
            # needed until the first scatter/node phase
            wna_s = load_const(wna[:], [D, D])
            wnb_s = load_const(wnb[:], [D, D])
            b1_s = load_const(b1[:], [D, 1], dt.float32)
            xt_s = load_const(xt_loc[:], [D, NPC])

            h_q = {}
            eh_q = {}
            ps2_q = {}
            xb_q = {}
            state = dict(an=None, b=None)
            blk_start_of = np.repeat(blk_start, n_chunks)

            def do_mm2(s2, kk):
                (h, off), ps2 = h_q[s2], ps2_q[s2]
                ksl = slice(kk * CHUNK, (kk + 1) * CHUNK)
                hsl = slice(off + kk * CHUNK, off + (kk + 1) * CHUNK)
                nc.tensor.matmul(ps2[:, ksl], h[:, hsl], w2_s[:],
                                 start=True, stop=skip_bias2)
                if not skip_bias2:
                    nc.tensor.matmul(ps2[:, ksl], ones_s[:], b2r_s[:],
                                     start=False, stop=True)

            def flush_node(t_now):
                if state.get("pend") and (t_now is None
                                          or t_now >= state["pend"][0] + 3):
                    _, b, an_ps, agg_sb, xb = state.pop("pend")
                    node_mlp(b, an_ps, agg_sb, xb)

            def do_scatter(s3, kk):
                eh = eh_q[s3]
                t = (SUPER // CHUNK) * s3 + kk
                flush_node(t)
                b = int(blk_of_chunk[t])
                nb_c = int(n_chunks[b])
                rel = t - int(blk_start_of[t])
                # two alternating accumulators (A/B halves of one packed PSUM
                # tile) break the PSUM-RAW accumulate chain; third slice is
                # the node-MLP accumulator for this block
                if rel == 0:
                    an_ps = psum.tile([128, BLK + D], dt.float32, tag="an_ps")
                    state["an"] = an_ps
                    n_w = min(BLK, NPC - b * BLK)
                    xb = npool.tile([128, D], dt.float32, tag="xb")
                    nc.sync.dma_start(out=xb[:n_w, :],
                                      in_=x_loc[b * BLK:b * BLK + n_w, :])
                    xb_q[b] = xb
                if rel == 1:
                    b_ps = psum.tile([128, BLK], dt.float32, tag="b_ps")
                    state["b"] = b_ps
                p = rel % 2
                acc = state["an"][:, 0:BLK] if p == 0 else state["b"][:]
                last_rel = nb_c - 1 - ((nb_c - 1 - p) % 2)
                bo, to = divmod(t, GBATCH // CHUNK)
                ot = gbufs[bo][2]
                ksl = slice(kk * CHUNK, (kk + 1) * CHUNK)
                nc.tensor.matmul(acc, eh[:, ksl],
                                 ot[:, to * CHUNK:(to + 1) * CHUNK],
                                 start=rel < 2, stop=rel == last_rel)
                if rel == nb_c - 1:
                    flush_node(None)   # never hold two pending blocks
                    agg_sb = npool.tile([128, BLK], cdt, tag="agg")
                    if nb_c >= 2:
                        # tensor ops may read only one PSUM operand: stage B
                        bsb = npool.tile([128, BLK], cdt, tag="bsb")
                        nc.vector.tensor_copy(bsb[:], state["b"][:])
                        nc.vector.tensor_add(agg_sb[:], state["an"][:, 0:BLK],
                                             bsb[:])
                    else:
                        nc.vector.tensor_copy(agg_sb[:], state["an"][:, 0:BLK])
                    # defer the node matmuls a few chunk slots so the PE
                    # never waits on the DVE merge
                    state["pend"] = (t, b, state["an"], agg_sb, xb_q.pop(b))

            def mm1a(sx, ps1):
                bi, k = divmod(sx, GBATCH // SUPER)
                gt = gbufs[bi][0]
                nc.tensor.matmul(ps1[:], w1a_s[:],
                                 gt[:, k * SUPER:(k + 1) * SUPER],
                                 start=True, stop=False)

            def mm1b(sx, ps1):
                bi, k = divmod(sx, GBATCH // SUPER)
                et, w = gbufs[bi][1], gbufs[bi][3]
                a, c0 = divmod(k * SUPER, w)
                nc.tensor.matmul(ps1[:],
                                 w1b_s[32 * a:32 * a + 32, :],
                                 et[32 * a:32 * a + 32, c0:c0 + SUPER],
                                 start=False, stop=True)

            for it in range(n_sup + 2):
                s1, s2, s3 = it, it - 1, it - 2
                # mm1 runs on PAIRS of supers (even iterations) so the w1a/w1b
                # stationaries are reused and the ps1 accumulate chains of the
                # two supers hide each other's latency
                em1 = s1 < n_sup and s1 % 2 == 0
                pw = 2 if s1 + 1 < n_sup else 1
                em2 = 0 <= s2 < n_sup
                em3 = s3 >= 0
                if em1:
                    bi, k = divmod(s1, GBATCH // SUPER)
                    if k == 0:
                        issue_batch(bi + 1)
                    if k == 4:
                        issue_batch(bi + 2)
                    ps1_l = []
                    for j in range(pw):
                        ps1j = psum.tile([128, SUPER], dt.float32, tag="ps1")
                        ps1_l.append(ps1j)
                        mm1a(s1 + j, ps1j)
                if em2:
                    ps2 = psum.tile([128, SUPER], dt.float32, tag="ps2")
                    ps2_q[s2] = ps2
                    do_mm2(s2, 0)
                if em3:
                    do_scatter(s3, 0)
                if em2:
                    do_mm2(s2, 1)
                if em3:
                    do_scatter(s3, 1)
                if em1:
                    for j in range(pw):
                        mm1b(s1 + j, ps1_l[j])
                    for j in range(pw):
                        h = hpool.tile([128, SUPER], cdt, tag="h")
                        nc.scalar.activation(h[:], ps1_l[j][:], relu,
                                             bias=b1_s[:])
                        h_q[s1 + j] = (h, 0)
                if em2:
                    do_mm2(s2, 2)
                if em3:
                    do_scatter(s3, 2)
                if em2:
                    do_mm2(s2, 3)
                if em3:
                    do_scatter(s3, 3)
                    eh_q.pop(s3)
                if em2:
                    h_q.pop(s2)
                    eh = epool.tile([128, SUPER], cdt, tag="eh")
                    if s2 % 8 == 0:
                        nc.scalar.activation(eh[:], ps2_q[s2][:], relu)
                    else:
                        nc.vector.tensor_scalar_max(eh[:], ps2_q[s2][:], 0.0)
                    ps2_q.pop(s2)
                    eh_q[s2] = eh
            flush_node(None)

    nc.compile()
    return nc


def _prepare(**inputs):
    x = np.ascontiguousarray(np.asarray(inputs["x"], F32))
    ei = np.asarray(inputs["edge_index"]).astype(np.int64)
    ea = np.ascontiguousarray(np.asarray(inputs["edge_attr"], F32))
    W_e1 = np.asarray(inputs["W_e1"], F32)
    b_e1 = np.asarray(inputs["b_e1"], F32)
    W_e2 = np.asarray(inputs["W_e2"], F32)
    b_e2 = np.asarray(inputs["b_e2"], F32)
    W_n = np.asarray(inputs["W_n"], F32)
    b_n = np.asarray(inputs["b_n"], F32)
    gamma = np.asarray(inputs["gamma"], F32)
    beta = np.asarray(inputs["beta"], F32)

    cnp = _np_cdt()
    dest, src = ei[0], ei[1]
    sched, per_core = _build_schedule(dest, src)
    sched["skip_bias2"] = bool(np.all(b_e2 == 0))
    sched["skip_biasn"] = bool(np.all(b_n == 0))
    sched["skip_affine"] = bool(np.all(gamma == 1) and np.all(beta == 0))
    nc = _build_bass(sched)

    import ml_dtypes
    L_tot = sched["L_tot"]
    T_tot = sched["T_tot"]
    ones_r = np.ones((1, 128), cnp)
    iota_d = np.arange(BLK, dtype=F32)
    gma = np.tile(gamma[None, :], (128, 1)).astype(F32)
    bta = np.tile(beta[None, :], (128, 1)).astype(F32)

    ea_z = np.concatenate([ea, np.zeros((1, EA), F32)], axis=0)  # -1 -> zeros
    x_cdt = x.astype(cnp)

    def pack_ea(ea_pad):
        """[L, 32] -> [64, L/2]: row 32a+d, col c = ea_pad[a*w + c, d] per
        GBATCH-edge batch of width w = g_n//2."""
        outs = []
        for e0 in range(0, L_tot, GBATCH):
            g_n = min(GBATCH, L_tot - e0)
            w = g_n // 2
            blk = ea_pad[e0:e0 + g_n].reshape(2, w, EA)
            outs.append(blk.transpose(0, 2, 1).reshape(2 * EA, w))
        return np.ascontiguousarray(np.concatenate(outs, axis=1))

    in_maps = []
    for c in range(NC):
        pc = per_core[c]
        gx = np.ascontiguousarray(x_cdt[pc["src"]].T)       # [128, L] bf16
        ea2 = pack_ea(ea_z[pc["ea_perm"]].astype(cnp))      # [64, L/2]
        dr = pc["dest_rel"].reshape(T_tot, CHUNK)           # [T, e]
        oh = (dr[:, :, None] == iota_d[None, None, :])      # [T, e, d]
        oh_t = np.ascontiguousarray(
            oh.transpose(1, 0, 2).reshape(CHUNK, L_tot)
        ).astype(ml_dtypes.float8_e4m3fn)
        xs = x[c * NPC:(c + 1) * NPC]
        in_maps.append({
            "gx_t": gx, "ea2": ea2, "oh_t": oh_t,
            "xt_loc": np.ascontiguousarray(xs.T.astype(cnp)),
            "x_loc": xs,
            "w1a": np.ascontiguousarray(W_e1[:D].astype(cnp)),
            "w1b2": np.ascontiguousarray(np.tile(W_e1[D:], (2, 1)).astype(cnp)),
            "w2": W_e2.astype(cnp),
            "wna": np.ascontiguousarray(W_n[:D].astype(cnp)),
            "wnb": np.ascontiguousarray(W_n[D:].astype(cnp)),
            "b1": b_e1[:, None].copy(),
            "b2r": b_e2[None, :].astype(cnp),
            "bnr": b_n[None, :].astype(cnp),
            "ones_r": ones_r, "gma": gma, "bta": bta,
        })
    return nc, in_maps


def kernel(**inputs) -> np.ndarray:
    nc, in_maps = _prepare(**inputs)
    from concourse.bass_utils import run_bass_kernel_spmd
    res = run_bass_kernel_spmd(nc, in_maps, list(range(NC)))
    return np.concatenate(
        [np.asarray(res.results[c]["out"]) for c in range(NC)],
        axis=0).astype(np.float32)


# revision 23
# speedup vs baseline: 1.0794x; 1.0224x over previous
"""DMPNN layer on 8 Trainium2 NeuronCores.

Sharding: edges are assigned to the core that owns their *destination* node
(50000 nodes / 8 cores = 6250 each), so the scatter-sum is core-local and no
collectives are needed.  The per-edge source-feature gather is done on the
HOST (same as the edge_attr permutation): each core receives a dense
feature-major stream gx_t = x[src].T for its (padded) edge list, so the
device never runs gpsimd dma_gather — the old bottleneck (~10ns/edge of Q7
descriptor generation, 905us total).

Within a core, edges are grouped by 128-node destination block (scatter-sum
is an accumulating onehot-matmul into one PSUM tile per block).  Per-block
chunk counts are maxed across cores so all 8 cores run the same static
program (SPMD); per-core variation is data only (gx/ea/dest_rel, padded with
dummy edges whose dest_rel=-1 masks them out of the scatter).

Datapath is bf16 (fp32 PSUM accumulation).  The main loop is software-
pipelined with a 2-super skew (mm1 of super s, mm2 of s-1, scatter of s-2)
so the PE never waits on the scalar-engine relus.
"""

import os

# The bass kernel executes through jax's axon/neuron platform.  A stray
# JAX_PLATFORMS=cpu (commonly set to keep jax off neuronxcc) would hide the
# NeuronCores, so drop it before jax is first imported.
if os.environ.get("JAX_PLATFORMS", "").strip() == "cpu":
    os.environ.pop("JAX_PLATFORMS")

import numpy as np

N_NODES = 50000
N_EDGES = 640000
D = 128          # node feature dim == hidden == output dim
EA = 32          # edge attr dim
NC = 8           # cores
NPC = N_NODES // NC   # nodes per core
BLK = 128        # node block width (scatter psum tile)
NB = (NPC + BLK - 1) // BLK   # 49 blocks per core (last one 106 nodes)
CHUNK = 128      # edge chunk (scatter/matmul granularity)
SUPER = 512      # edge super-chunk (mm1/relu batching)
GBATCH = 4096    # edges per DMA batch (1MB bf16 -> near-peak HBM bw)
EPS = 1e-5

F32 = np.float32


def _np_cdt():
    import ml_dtypes
    return ml_dtypes.bfloat16


def _build_schedule(dest: np.ndarray, src: np.ndarray):
    """Renumber destination nodes into balanced (core, block) bins (degree-
    sorted round-robin deal), then group edges by bin; the host inverts the
    renumbering when assembling the output.  Balancing makes the per-block
    edge counts nearly equal across cores, so the shared (max-over-cores)
    chunk schedule carries ~7% less padding than contiguous sharding."""
    deg = np.bincount(dest, minlength=N_NODES)
    order_d = np.argsort(-deg, kind="stable")
    n_slots = NC * NB                   # 392 bins; last block per core = 106
    small = NPC - (NB - 1) * BLK        # 106
    full_slots = np.array([s for s in range(n_slots) if s % NB != NB - 1])
    small_slots = np.array([s for s in range(n_slots) if s % NB == NB - 1])

    core_d = np.empty(N_NODES, np.int64)
    block_d = np.empty(N_NODES, np.int64)
    rel_d = np.empty(N_NODES, np.int64)
    # phase 1: `small` snake rounds over all 392 slots
    n1 = small * n_slots
    p1 = order_d[:n1].reshape(small, n_slots)
    slot_order = np.arange(n_slots)
    for r in range(small):
        row = p1[r] if r % 2 == 0 else p1[r][::-1]
        core_d[row] = slot_order // NB
        block_d[row] = slot_order % NB
        rel_d[row] = r
    # phase 2: remaining rounds over the 384 full slots
    p2 = order_d[n1:].reshape(BLK - small, len(full_slots))
    for r in range(BLK - small):
        row = p2[r] if r % 2 == 0 else p2[r][::-1]
        core_d[row] = full_slots // NB
        block_d[row] = full_slots % NB
        rel_d[row] = small + r

    # per-core node permutation: slot-major, rel-minor
    perm_nodes = np.empty((NC, NPC), np.int64)
    slot_of_node = core_d * NPC + block_d * BLK + rel_d
    inv = np.argsort(slot_of_node)
    perm_nodes = inv.reshape(NC, NPC)

    e_core = core_d[dest]
    e_block = block_d[dest]
    e_rel = rel_d[dest]
    key = e_core * NB + e_block
    order = np.argsort(key, kind="stable")
    key_s = key[order]
    cnt = np.bincount(key, minlength=NC * NB).reshape(NC, NB)

    # shared chunk counts per block: max over cores, >= 1; total padded to
    # 8 chunks (SUPER multiple AND 1024-edge alignment so every DMA batch's
    # ea2 half-width w is a multiple of SUPER — mm1b's group arithmetic
    # requires supers not to straddle the packed a-group boundary)
    n_chunks = np.maximum(1, -(-cnt.max(axis=0) // CHUNK))  # [NB]
    extra = (-int(n_chunks.sum())) % (2 * SUPER // CHUNK)
    n_chunks[NB - 1] += extra
    T_tot = int(n_chunks.sum())
    L_tot = T_tot * CHUNK

    # padded start offset of each block group within a core's stream
    pad_start = np.concatenate([[0], np.cumsum(n_chunks)[:-1]]) * CHUNK

    # rank of each edge within its (core, block) group
    grp_start = np.zeros(NC * NB + 1, np.int64)
    np.cumsum(np.bincount(key, minlength=NC * NB), out=grp_start[1:])
    rank = np.arange(N_EDGES) - grp_start[key_s]

    c_s = key_s // NB
    pos = pad_start[key_s % NB] + rank

    blk_of_chunk = np.repeat(np.arange(NB), n_chunks)

    per_core = []
    rel_s = e_rel[order]
    src_s = src[order]
    for c in range(NC):
        m = c_s == c
        p = pos[m]
        src_pad = np.zeros(L_tot, np.int64)
        src_pad[p] = src_s[m]
        dest_rel = np.full(L_tot, -1.0, F32)
        dest_rel[p] = rel_s[m].astype(F32)
        assert dest_rel.max() < BLK and (dest_rel[p] >= 0).all()
        ea_perm = np.full(L_tot, -1, np.int64)
        ea_perm[p] = order[m]   # original edge id per padded slot (-1 = dummy)
        per_core.append(dict(src=src_pad, dest_rel=dest_rel, ea_perm=ea_perm))

    sched = dict(n_chunks=n_chunks, T_tot=T_tot, L_tot=L_tot,
                 blk_of_chunk=blk_of_chunk, perm_nodes=perm_nodes)
    return sched, per_core


def _build_bass(sched):
    import concourse.bacc as bacc
    import concourse.mybir as mybir
    import concourse.tile as tile

    dt = mybir.dt
    cdt = dt.bfloat16
    T_tot = sched["T_tot"]
    L_tot = sched["L_tot"]
    n_chunks = sched["n_chunks"]
    blk_of_chunk = sched["blk_of_chunk"]
    skip_bias2 = sched["skip_bias2"]
    skip_biasn = sched["skip_biasn"]
    skip_affine = sched["skip_affine"]

    n_sup = T_tot // (SUPER // CHUNK)
    n_batches = -(-L_tot // GBATCH)

    # first/last chunk of each block
    blk_start = np.concatenate([[0], np.cumsum(n_chunks)[:-1]])
    first_of = np.zeros(T_tot, bool)
    last_of = np.zeros(T_tot, bool)
    first_of[blk_start] = True
    last_of[blk_start + n_chunks - 1] = True

    nc = bacc.Bacc("TRN2", target_bir_lowering=False, debug=False,
                   num_devices=NC)

    def din(name, shape, d=None):
        return nc.dram_tensor(name, shape, d or cdt, kind="ExternalInput").ap()

    gx_t = din("gx_t", [D, L_tot])
    ea2 = din("ea2", [2 * EA, L_tot // 2])
    oh_t = din("oh_t", [CHUNK, L_tot], dt.float8e4)  # host-built onehots
    xt_loc = din("xt_loc", [D, NPC])
    x_loc = din("x_loc", [NPC, D], dt.float32)
    w1a = din("w1a", [D, D])
    w1b2 = din("w1b2", [2 * EA, D])   # w1b replicated at partitions 0/32
    w2 = din("w2", [D, D])
    wna = din("wna", [D, D])
    wnb = din("wnb", [D, D])
    b1 = din("b1", [D, 1], dt.float32)
    b2r = din("b2r", [1, D])
    bnr = din("bnr", [1, D])
    ones_r = din("ones_r", [1, 128])
    gma = din("gma", [128, D], dt.float32)
    bta = din("bta", [128, D], dt.float32)
    out = nc.dram_tensor("out", [NPC, D], cdt, kind="ExternalOutput").ap()

    with tile.TileContext(nc) as tc:
        from contextlib import ExitStack
        ctx = ExitStack()
        with ctx:
            const = ctx.enter_context(tc.tile_pool(name="const", bufs=1))
            gpool = ctx.enter_context(tc.tile_pool(name="gx", bufs=3))
            eapool = ctx.enter_context(tc.tile_pool(name="ea", bufs=3))
            hpool = ctx.enter_context(tc.tile_pool(name="h", bufs=3))
            epool = ctx.enter_context(tc.tile_pool(name="eh", bufs=3))
            ohpool = ctx.enter_context(tc.tile_pool(name="ohp", bufs=4))
            psum = ctx.enter_context(tc.tile_pool(name="psum", bufs=2,
                                                  space="PSUM"))
            npool = ctx.enter_context(tc.tile_pool(name="node", bufs=3))

            def load_const(ap, shape, d=None):
                t = const.tile(shape, d or cdt, tag=f"c_{ap.tensor.name}")
                nc.sync.dma_start(out=t[:], in_=ap)
                return t

            w1a_s = load_const(w1a[:], [D, D])
            w1b_s = load_const(w1b2[:], [2 * EA, D])
            w2_s = load_const(w2[:], [D, D])
            if not skip_bias2:
                b2r_s = load_const(b2r[:], [1, D])
            if not (skip_bias2 and skip_biasn):
                ones_s = load_const(ones_r[:], [1, 128])
            if not skip_biasn:
                bnr_s = load_const(bnr[:], [1, D])
            if not skip_affine:
                gma_s = load_const(gma[:], [128, D], dt.float32)
                bta_s = load_const(bta[:], [128, D], dt.float32)

            eps_t = const.tile([128, 1], dt.float32, tag="eps")
            nc.vector.memset(eps_t[:], EPS)

            relu = mybir.ActivationFunctionType.Relu

            def node_mlp(b, an_ps, agg_sb, xb):
                """node MLP + residual layernorm for block b, consuming its
                merged scatter accumulator (SBUF bf16)."""
                n_w = min(BLK, NPC - b * BLK)
                cols = slice(b * BLK, b * BLK + n_w)
                ps_nn = an_ps[:, BLK:BLK + D]
                nc.tensor.matmul(ps_nn[:n_w, :], xt_s[:, cols], wna_s[:],
                                 start=True, stop=False)
                nc.tensor.matmul(ps_nn[:n_w, :], agg_sb[:, :n_w], wnb_s[:],
                                 start=False, stop=skip_biasn)
                if not skip_biasn:
                    nc.tensor.matmul(ps_nn[:n_w, :], ones_s[:1, :n_w],
                                     bnr_s[:], start=False, stop=True)
                o_sb = npool.tile([128, D], dt.float32, tag="o_sb")
                nc.scalar.activation(o_sb[:n_w, :], ps_nn[:n_w, :], relu)
                r_sb = npool.tile([128, D], cdt, tag="r_sb")
                nc.vector.tensor_add(r_sb[:n_w, :], o_sb[:n_w, :], xb[:n_w, :])
                # layernorm over free dim
                st6 = npool.tile([128, 6], dt.float32, tag="st6")
                nc.vector.bn_stats(st6[:n_w, :], r_sb[:n_w, :])
                mv = npool.tile([128, 2], dt.float32, tag="mv")
                nc.vector.bn_aggr(mv[:n_w, :], st6[:n_w, :])
                sd = npool.tile([128, 1], dt.float32, tag="sd")
                nc.scalar.activation(sd[:n_w, :], mv[:n_w, 1:2],
                                     mybir.ActivationFunctionType.Sqrt,
                                     bias=eps_t[:n_w, :])
                rstd = npool.tile([128, 1], dt.float32, tag="rstd")
                nc.vector.reciprocal(rstd[:n_w, :], sd[:n_w, :])
                y = npool.tile([128, D], cdt, tag="y")
                nc.vector.tensor_scalar(y[:n_w, :], r_sb[:n_w, :],
                                        mv[:n_w, 0:1], rstd[:n_w, :],
                                        op0=mybir.AluOpType.subtract,
                                        op1=mybir.AluOpType.mult)
                if not skip_affine:
                    y2 = npool.tile([128, D], dt.float32, tag="y2")
                    nc.vector.tensor_mul(y2[:n_w, :], y[:n_w, :],
                                         gma_s[:n_w, :])
                    y3 = npool.tile([128, D], cdt, tag="y3")
                    nc.vector.tensor_add(y3[:n_w, :], y2[:n_w, :],
                                         bta_s[:n_w, :])
                    y = y3
                nc.sync.dma_start(out=out[b * BLK:b * BLK + n_w, :],
                                  in_=y[:n_w, :])

            # ---------------- edge phase (2-super pipeline skew) -----------
            gbufs = {}

            def issue_batch(bi):
                if bi >= n_batches or bi in gbufs:
                    return
                e0 = bi * GBATCH
                g_n = min(GBATCH, L_tot - e0)
                w = g_n // 2
                gt = gpool.tile([128, GBATCH], cdt, tag="gbuf")
                nc.sync.dma_start(out=gt[:, :g_n], in_=gx_t[:, e0:e0 + g_n])
                et = eapool.tile([2 * EA, GBATCH // 2], cdt, tag="eab")
                nc.sync.dma_start(out=et[:, :w],
                                  in_=ea2[:, e0 // 2:e0 // 2 + w])
                ot = ohpool.tile([CHUNK, GBATCH], dt.float8e4, tag="ohb")
                nc.sync.dma_start(out=ot[:, :g_n], in_=oh_t[:, e0:e0 + g_n])
                gbufs[bi] = (gt, et, ot, w)

            issue_batch(0)
            issue_batch(1)
            # heavier consts load behind the first edge batches; none are
            # needed until the first scatter/node phase
            wna_s = load_const(wna[:], [D, D])
            wnb_s = load_const(wnb[:], [D, D])
            b1_s = load_const(b1[:], [D, 1], dt.float32)
            xt_s = load_const(xt_loc[:], [D, NPC])

            h_q = {}
            eh_q = {}
            ps2_q = {}
            xb_q = {}
            state = dict(an=None, b=None)
            blk_start_of = np.repeat(blk_start, n_chunks)

            def do_mm2(s2, kk):
                (h, off), ps2 = h_q[s2], ps2_q[s2]
                ksl = slice(kk * CHUNK, (kk + 1) * CHUNK)
                hsl = slice(off + kk * CHUNK, off + (kk + 1) * CHUNK)
                nc.tensor.matmul(ps2[:, ksl], h[:, hsl], w2_s[:],
                                 start=True, stop=skip_bias2)
                if not skip_bias2:
                    nc.tensor.matmul(ps2[:, ksl], ones_s[:], b2r_s[:],
                                     start=False, stop=True)

            def flush_node(t_now):
                if state.get("pend") and (t_now is None
                                          or t_now >= state["pend"][0] + 3):
                    _, b, an_ps, agg_sb, xb = state.pop("pend")
                    node_mlp(b, an_ps, agg_sb, xb)

            def do_scatter(s3, kk):
                eh = eh_q[s3]
                t = (SUPER // CHUNK) * s3 + kk
                flush_node(t)
                b = int(blk_of_chunk[t])
                nb_c = int(n_chunks[b])
                rel = t - int(blk_start_of[t])
                # two alternating accumulators (A/B halves of one packed PSUM
                # tile) break the PSUM-RAW accumulate chain; third slice is
                # the node-MLP accumulator for this block
                if rel == 0:
                    an_ps = psum.tile([128, BLK + D], dt.float32, tag="an_ps")
                    state["an"] = an_ps
                    n_w = min(BLK, NPC - b * BLK)
                    xb = npool.tile([128, D], dt.float32, tag="xb")
                    nc.sync.dma_start(out=xb[:n_w, :],
                                      in_=x_loc[b * BLK:b * BLK + n_w, :])
                    xb_q[b] = xb
                if rel == 1:
                    b_ps = psum.tile([128, BLK], dt.float32, tag="b_ps")
                    state["b"] = b_ps
                p = rel % 2
                acc = state["an"][:, 0:BLK] if p == 0 else state["b"][:]
                last_rel = nb_c - 1 - ((nb_c - 1 - p) % 2)
                bo, to = divmod(t, GBATCH // CHUNK)
                ot = gbufs[bo][2]
                ksl = slice(kk * CHUNK, (kk + 1) * CHUNK)
                nc.tensor.matmul(acc, eh[:, ksl],
                                 ot[:, to * CHUNK:(to + 1) * CHUNK],
                                 start=rel < 2, stop=rel == last_rel)
                if rel == nb_c - 1:
                    flush_node(None)   # never hold two pending blocks
                    agg_sb = npool.tile([128, BLK], cdt, tag="agg")
                    if nb_c >= 2:
                        # tensor ops may read only one PSUM operand: stage B
                        bsb = npool.tile([128, BLK], cdt, tag="bsb")
                        nc.vector.tensor_copy(bsb[:], state["b"][:])
                        nc.vector.tensor_add(agg_sb[:], state["an"][:, 0:BLK],
                                             bsb[:])
                    else:
                        nc.vector.tensor_copy(agg_sb[:], state["an"][:, 0:BLK])
                    # defer the node matmuls a few chunk slots so the PE
                    # never waits on the DVE merge
                    state["pend"] = (t, b, state["an"], agg_sb, xb_q.pop(b))

            def mm1a(sx, ps1):
                bi, k = divmod(sx, GBATCH // SUPER)
                gt = gbufs[bi][0]
                nc.tensor.matmul(ps1[:], w1a_s[:],
                                 gt[:, k * SUPER:(k + 1) * SUPER],
                                 start=True, stop=False)

            def mm1b(sx, ps1):
                bi, k = divmod(sx, GBATCH // SUPER)
                et, w = gbufs[bi][1], gbufs[bi][3]
                a, c0 = divmod(k * SUPER, w)
                nc.tensor.matmul(ps1[:],
                                 w1b_s[32 * a:32 * a + 32, :],
                                 et[32 * a:32 * a + 32, c0:c0 + SUPER],
                                 start=False, stop=True)

            for it in range(n_sup + 2):
                s1, s2, s3 = it, it - 1, it - 2
                # mm1 runs on PAIRS of supers (even iterations) so the w1a/w1b
                # stationaries are reused and the ps1 accumulate chains of the
                # two supers hide each other's latency
                em1 = s1 < n_sup and s1 % 2 == 0
                pw = 2 if s1 + 1 < n_sup else 1
                em2 = 0 <= s2 < n_sup
                em3 = s3 >= 0
                if em1:
                    bi, k = divmod(s1, GBATCH // SUPER)
                    if k == 0:
                        issue_batch(bi + 1)
                    if k == 4:
                        issue_batch(bi + 2)
                    ps1_l = []
                    for j in range(pw):
                        ps1j = psum.tile([128, SUPER], dt.float32, tag="ps1")
                        ps1_l.append(ps1j)
                        mm1a(s1 + j, ps1j)
                if em2:
                    ps2 = psum.tile([128, SUPER], dt.float32, tag="ps2")
                    ps2_q[s2] = ps2
                    do_mm2(s2, 0)
                if em3:
                    do_scatter(s3, 0)
                if em2:
                    do_mm2(s2, 1)
                if em3:
                    do_scatter(s3, 1)
                if em1:
                    for j in range(pw):
                        mm1b(s1 + j, ps1_l[j])
                    for j in range(pw):
                        h = hpool.tile([128, SUPER], cdt, tag="h")
                        nc.scalar.activation(h[:], ps1_l[j][:], relu,
                                             bias=b1_s[:])
                        h_q[s1 + j] = (h, 0)
                if em2:
                    do_mm2(s2, 2)
                if em3:
                    do_scatter(s3, 2)
                if em2:
                    do_mm2(s2, 3)
                if em3:
                    do_scatter(s3, 3)
                    eh_q.pop(s3)
                if em2:
                    h_q.pop(s2)
                    eh = epool.tile([128, SUPER], cdt, tag="eh")
                    if s2 % 8 == 0:
                        nc.scalar.activation(eh[:], ps2_q[s2][:], relu)
                    else:
                        nc.vector.tensor_scalar_max(eh[:], ps2_q[s2][:], 0.0)
                    ps2_q.pop(s2)
                    eh_q[s2] = eh
            flush_node(None)

    nc.compile()
    return nc


def _prepare(**inputs):
    x = np.ascontiguousarray(np.asarray(inputs["x"], F32))
    ei = np.asarray(inputs["edge_index"]).astype(np.int64)
    ea = np.ascontiguousarray(np.asarray(inputs["edge_attr"], F32))
    W_e1 = np.asarray(inputs["W_e1"], F32)
    b_e1 = np.asarray(inputs["b_e1"], F32)
    W_e2 = np.asarray(inputs["W_e2"], F32)
    b_e2 = np.asarray(inputs["b_e2"], F32)
    W_n = np.asarray(inputs["W_n"], F32)
    b_n = np.asarray(inputs["b_n"], F32)
    gamma = np.asarray(inputs["gamma"], F32)
    beta = np.asarray(inputs["beta"], F32)

    cnp = _np_cdt()
    dest, src = ei[0], ei[1]
    sched, per_core = _build_schedule(dest, src)
    sched["skip_bias2"] = bool(np.all(b_e2 == 0))
    sched["skip_biasn"] = bool(np.all(b_n == 0))
    sched["skip_affine"] = bool(np.all(gamma == 1) and np.all(beta == 0))
    nc = _build_bass(sched)

    import ml_dtypes
    L_tot = sched["L_tot"]
    T_tot = sched["T_tot"]
    ones_r = np.ones((1, 128), cnp)
    iota_d = np.arange(BLK, dtype=F32)
    gma = np.tile(gamma[None, :], (128, 1)).astype(F32)
    bta = np.tile(beta[None, :], (128, 1)).astype(F32)

    ea_z = np.concatenate([ea, np.zeros((1, EA), F32)], axis=0)  # -1 -> zeros
    x_cdt = x.astype(cnp)

    def pack_ea(ea_pad):
        """[L, 32] -> [64, L/2]: row 32a+d, col c = ea_pad[a*w + c, d] per
        GBATCH-edge batch of width w = g_n//2."""
        outs = []
        for e0 in range(0, L_tot, GBATCH):
            g_n = min(GBATCH, L_tot - e0)
            w = g_n // 2
            blk = ea_pad[e0:e0 + g_n].reshape(2, w, EA)
            outs.append(blk.transpose(0, 2, 1).reshape(2 * EA, w))
        return np.ascontiguousarray(np.concatenate(outs, axis=1))

    in_maps = []
    for c in range(NC):
        pc = per_core[c]
        gx = np.ascontiguousarray(x_cdt[pc["src"]].T)       # [128, L] bf16
        ea2 = pack_ea(ea_z[pc["ea_perm"]].astype(cnp))      # [64, L/2]
        dr = pc["dest_rel"].reshape(T_tot, CHUNK)           # [T, e]
        oh = (dr[:, :, None] == iota_d[None, None, :])      # [T, e, d]
        oh_t = np.ascontiguousarray(
            oh.transpose(1, 0, 2).reshape(CHUNK, L_tot)
        ).astype(ml_dtypes.float8_e4m3fn)
        xs = np.ascontiguousarray(x[sched["perm_nodes"][c]])
        in_maps.append({
            "gx_t": gx, "ea2": ea2, "oh_t": oh_t,
            "xt_loc": np.ascontiguousarray(xs.T.astype(cnp)),
            "x_loc": xs,
            "w1a": np.ascontiguousarray(W_e1[:D].astype(cnp)),
            "w1b2": np.ascontiguousarray(np.tile(W_e1[D:], (2, 1)).astype(cnp)),
            "w2": W_e2.astype(cnp),
            "wna": np.ascontiguousarray(W_n[:D].astype(cnp)),
            "wnb": np.ascontiguousarray(W_n[D:].astype(cnp)),
            "b1": b_e1[:, None].copy(),
            "b2r": b_e2[None, :].astype(cnp),
            "bnr": b_n[None, :].astype(cnp),
            "ones_r": ones_r, "gma": gma, "bta": bta,
        })
    return nc, in_maps, sched["perm_nodes"]


def kernel(**inputs) -> np.ndarray:
    nc, in_maps, perm_nodes = _prepare(**inputs)
    from concourse.bass_utils import run_bass_kernel_spmd
    res = run_bass_kernel_spmd(nc, in_maps, list(range(NC)))
    out = np.empty((N_NODES, D), np.float32)
    for c in range(NC):
        out[perm_nodes[c]] = np.asarray(res.results[c]["out"]).astype(np.float32)
    return out


# revision 24
# speedup vs baseline: 1.0877x; 1.0076x over previous
"""DMPNN layer on 8 Trainium2 NeuronCores.

Sharding: edges are assigned to the core that owns their *destination* node
(50000 nodes / 8 cores = 6250 each), so the scatter-sum is core-local and no
collectives are needed.  The per-edge source-feature gather is done on the
HOST (same as the edge_attr permutation): each core receives a dense
feature-major stream gx_t = x[src].T for its (padded) edge list, so the
device never runs gpsimd dma_gather — the old bottleneck (~10ns/edge of Q7
descriptor generation, 905us total).

Within a core, edges are grouped by 128-node destination block (scatter-sum
is an accumulating onehot-matmul into one PSUM tile per block).  Per-block
chunk counts are maxed across cores so all 8 cores run the same static
program (SPMD); per-core variation is data only (gx/ea/dest_rel, padded with
dummy edges whose dest_rel=-1 masks them out of the scatter).

Datapath is bf16 (fp32 PSUM accumulation).  The main loop is software-
pipelined with a 2-super skew (mm1 of super s, mm2 of s-1, scatter of s-2)
so the PE never waits on the scalar-engine relus.
"""

import os

# The bass kernel executes through jax's axon/neuron platform.  A stray
# JAX_PLATFORMS=cpu (commonly set to keep jax off neuronxcc) would hide the
# NeuronCores, so drop it before jax is first imported.
if os.environ.get("JAX_PLATFORMS", "").strip() == "cpu":
    os.environ.pop("JAX_PLATFORMS")

import numpy as np

N_NODES = 50000
N_EDGES = 640000
D = 128          # node feature dim == hidden == output dim
EA = 32          # edge attr dim
NC = 8           # cores
NPC = N_NODES // NC   # nodes per core
BLK = 128        # node block width (scatter psum tile)
NB = (NPC + BLK - 1) // BLK   # 49 blocks per core (last one 106 nodes)
CHUNK = 128      # edge chunk (scatter/matmul granularity)
SUPER = 512      # edge super-chunk (mm1/relu batching)
GBATCH = 4096    # edges per DMA batch (1MB bf16 -> near-peak HBM bw)
EPS = 1e-5

F32 = np.float32


def _np_cdt():
    import ml_dtypes
    return ml_dtypes.bfloat16


def _build_schedule(dest: np.ndarray, src: np.ndarray):
    """Renumber destination nodes into balanced (core, block) bins (degree-
    sorted round-robin deal), then group edges by bin; the host inverts the
    renumbering when assembling the output.  Balancing makes the per-block
    edge counts nearly equal across cores, so the shared (max-over-cores)
    chunk schedule carries ~7% less padding than contiguous sharding."""
    deg = np.bincount(dest, minlength=N_NODES)
    order_d = np.argsort(-deg, kind="stable")
    n_slots = NC * NB                   # 392 bins; last block per core = 106
    small = NPC - (NB - 1) * BLK        # 106
    full_slots = np.array([s for s in range(n_slots) if s % NB != NB - 1])
    small_slots = np.array([s for s in range(n_slots) if s % NB == NB - 1])

    core_d = np.empty(N_NODES, np.int64)
    block_d = np.empty(N_NODES, np.int64)
    rel_d = np.empty(N_NODES, np.int64)
    # phase 1: `small` snake rounds over all 392 slots
    n1 = small * n_slots
    p1 = order_d[:n1].reshape(small, n_slots)
    slot_order = np.arange(n_slots)
    for r in range(small):
        row = p1[r] if r % 2 == 0 else p1[r][::-1]
        core_d[row] = slot_order // NB
        block_d[row] = slot_order % NB
        rel_d[row] = r
    # phase 2: remaining rounds over the 384 full slots
    p2 = order_d[n1:].reshape(BLK - small, len(full_slots))
    for r in range(BLK - small):
        row = p2[r] if r % 2 == 0 else p2[r][::-1]
        core_d[row] = full_slots // NB
        block_d[row] = full_slots % NB
        rel_d[row] = small + r

    # per-core node permutation: slot-major, rel-minor
    perm_nodes = np.empty((NC, NPC), np.int64)
    slot_of_node = core_d * NPC + block_d * BLK + rel_d
    inv = np.argsort(slot_of_node)
    perm_nodes = inv.reshape(NC, NPC)

    e_core = core_d[dest]
    e_block = block_d[dest]
    e_rel = rel_d[dest]
    key = e_core * NB + e_block
    order = np.argsort(key, kind="stable")
    key_s = key[order]
    cnt = np.bincount(key, minlength=NC * NB).reshape(NC, NB)

    # shared chunk counts per block: max over cores, >= 1; total padded to
    # 8 chunks (SUPER multiple AND 1024-edge alignment so every DMA batch's
    # ea2 half-width w is a multiple of SUPER — mm1b's group arithmetic
    # requires supers not to straddle the packed a-group boundary)
    n_chunks = np.maximum(1, -(-cnt.max(axis=0) // CHUNK))  # [NB]
    extra = (-int(n_chunks.sum())) % (2 * SUPER // CHUNK)
    n_chunks[NB - 1] += extra
    T_tot = int(n_chunks.sum())
    L_tot = T_tot * CHUNK

    # padded start offset of each block group within a core's stream
    pad_start = np.concatenate([[0], np.cumsum(n_chunks)[:-1]]) * CHUNK

    # rank of each edge within its (core, block) group
    grp_start = np.zeros(NC * NB + 1, np.int64)
    np.cumsum(np.bincount(key, minlength=NC * NB), out=grp_start[1:])
    rank = np.arange(N_EDGES) - grp_start[key_s]

    c_s = key_s // NB
    pos = pad_start[key_s % NB] + rank

    blk_of_chunk = np.repeat(np.arange(NB), n_chunks)

    per_core = []
    rel_s = e_rel[order]
    src_s = src[order]
    for c in range(NC):
        m = c_s == c
        p = pos[m]
        src_pad = np.zeros(L_tot, np.int64)
        src_pad[p] = src_s[m]
        dest_rel = np.full(L_tot, -1.0, F32)
        dest_rel[p] = rel_s[m].astype(F32)
        assert dest_rel.max() < BLK and (dest_rel[p] >= 0).all()
        ea_perm = np.full(L_tot, -1, np.int64)
        ea_perm[p] = order[m]   # original edge id per padded slot (-1 = dummy)
        per_core.append(dict(src=src_pad, dest_rel=dest_rel, ea_perm=ea_perm))

    sched = dict(n_chunks=n_chunks, T_tot=T_tot, L_tot=L_tot,
                 blk_of_chunk=blk_of_chunk, perm_nodes=perm_nodes)
    return sched, per_core


def _build_bass(sched):
    import concourse.bacc as bacc
    import concourse.mybir as mybir
    import concourse.tile as tile

    dt = mybir.dt
    cdt = dt.bfloat16
    T_tot = sched["T_tot"]
    L_tot = sched["L_tot"]
    n_chunks = sched["n_chunks"]
    blk_of_chunk = sched["blk_of_chunk"]
    skip_bias2 = sched["skip_bias2"]
    skip_biasn = sched["skip_biasn"]
    skip_affine = sched["skip_affine"]

    n_sup = T_tot // (SUPER // CHUNK)
    n_batches = -(-L_tot // GBATCH)

    # first/last chunk of each block
    blk_start = np.concatenate([[0], np.cumsum(n_chunks)[:-1]])
    first_of = np.zeros(T_tot, bool)
    last_of = np.zeros(T_tot, bool)
    first_of[blk_start] = True
    last_of[blk_start + n_chunks - 1] = True

    nc = bacc.Bacc("TRN2", target_bir_lowering=False, debug=False,
                   num_devices=NC)

    def din(name, shape, d=None):
        return nc.dram_tensor(name, shape, d or cdt, kind="ExternalInput").ap()

    gx_t = din("gx_t", [D, L_tot])
    ea2 = din("ea2", [2 * EA, L_tot // 2])
    oh_t = din("oh_t", [CHUNK, L_tot], dt.float8e4)  # host-built onehots
    xt_loc = din("xt_loc", [D, NPC])
    x_loc = din("x_loc", [NPC, D], dt.float32)
    w1a = din("w1a", [D, D])
    w1b2 = din("w1b2", [2 * EA, D])   # w1b replicated at partitions 0/32
    w2 = din("w2", [D, D])
    wna = din("wna", [D, D])
    wnb = din("wnb", [D, D])
    b1 = din("b1", [D, 1], dt.float32)
    b2r = din("b2r", [1, D])
    bnr = din("bnr", [1, D])
    ones_r = din("ones_r", [1, 128])
    gma = din("gma", [128, D], dt.float32)
    bta = din("bta", [128, D], dt.float32)
    out = nc.dram_tensor("out", [NPC, D], cdt, kind="ExternalOutput").ap()

    with tile.TileContext(nc) as tc:
        from contextlib import ExitStack
        ctx = ExitStack()
        with ctx:
            const = ctx.enter_context(tc.tile_pool(name="const", bufs=1))
            gpool = ctx.enter_context(tc.tile_pool(name="gx", bufs=3))
            eapool = ctx.enter_context(tc.tile_pool(name="ea", bufs=3))
            hpool = ctx.enter_context(tc.tile_pool(name="h", bufs=3))
            epool = ctx.enter_context(tc.tile_pool(name="eh", bufs=3))
            ohpool = ctx.enter_context(tc.tile_pool(name="ohp", bufs=4))
            psum = ctx.enter_context(tc.tile_pool(name="psum", bufs=2,
                                                  space="PSUM"))
            npool = ctx.enter_context(tc.tile_pool(name="node", bufs=3))

            def load_const(ap, shape, d=None):
                t = const.tile(shape, d or cdt, tag=f"c_{ap.tensor.name}")
                nc.sync.dma_start(out=t[:], in_=ap)
                return t

            w1a_s = load_const(w1a[:], [D, D])
            w1b_s = load_const(w1b2[:], [2 * EA, D])
            w2_s = load_const(w2[:], [D, D])
            if not skip_bias2:
                b2r_s = load_const(b2r[:], [1, D])
            if not (skip_bias2 and skip_biasn):
                ones_s = load_const(ones_r[:], [1, 128])
            if not skip_biasn:
                bnr_s = load_const(bnr[:], [1, D])
            if not skip_affine:
                gma_s = load_const(gma[:], [128, D], dt.float32)
                bta_s = load_const(bta[:], [128, D], dt.float32)

            eps_t = const.tile([128, 1], dt.float32, tag="eps")
            nc.vector.memset(eps_t[:], EPS)

            relu = mybir.ActivationFunctionType.Relu

            def node_mlp(b, abn_ps, agg_sb, xb):
                """node MLP + residual layernorm for block b, consuming its
                merged scatter accumulator (SBUF bf16).  start=False: the N
                columns' has_written bits are still clear from the block's
                bank start, so the first matmul overwrites."""
                n_w = min(BLK, NPC - b * BLK)
                cols = slice(b * BLK, b * BLK + n_w)
                ps_nn = abn_ps[:, 2 * BLK:2 * BLK + D]
                nc.tensor.matmul(ps_nn[:n_w, :], xt_s[:, cols], wna_s[:],
                                 start=False, stop=False)
                nc.tensor.matmul(ps_nn[:n_w, :], agg_sb[:, :n_w], wnb_s[:],
                                 start=False, stop=skip_biasn)
                if not skip_biasn:
                    nc.tensor.matmul(ps_nn[:n_w, :], ones_s[:1, :n_w],
                                     bnr_s[:], start=False, stop=True)
                o_sb = npool.tile([128, D], dt.float32, tag="o_sb")
                nc.scalar.activation(o_sb[:n_w, :], ps_nn[:n_w, :], relu)
                r_sb = npool.tile([128, D], cdt, tag="r_sb")
                nc.vector.tensor_add(r_sb[:n_w, :], o_sb[:n_w, :], xb[:n_w, :])
                # layernorm over free dim
                st6 = npool.tile([128, 6], dt.float32, tag="st6")
                nc.vector.bn_stats(st6[:n_w, :], r_sb[:n_w, :])
                mv = npool.tile([128, 2], dt.float32, tag="mv")
                nc.vector.bn_aggr(mv[:n_w, :], st6[:n_w, :])
                sd = npool.tile([128, 1], dt.float32, tag="sd")
                nc.scalar.activation(sd[:n_w, :], mv[:n_w, 1:2],
                                     mybir.ActivationFunctionType.Sqrt,
                                     bias=eps_t[:n_w, :])
                rstd = npool.tile([128, 1], dt.float32, tag="rstd")
                nc.vector.reciprocal(rstd[:n_w, :], sd[:n_w, :])
                y = npool.tile([128, D], cdt, tag="y")
                nc.vector.tensor_scalar(y[:n_w, :], r_sb[:n_w, :],
                                        mv[:n_w, 0:1], rstd[:n_w, :],
                                        op0=mybir.AluOpType.subtract,
                                        op1=mybir.AluOpType.mult)
                if not skip_affine:
                    y2 = npool.tile([128, D], dt.float32, tag="y2")
                    nc.vector.tensor_mul(y2[:n_w, :], y[:n_w, :],
                                         gma_s[:n_w, :])
                    y3 = npool.tile([128, D], cdt, tag="y3")
                    nc.vector.tensor_add(y3[:n_w, :], y2[:n_w, :],
                                         bta_s[:n_w, :])
                    y = y3
                nc.sync.dma_start(out=out[b * BLK:b * BLK + n_w, :],
                                  in_=y[:n_w, :])

            # ---------------- edge phase (2-super pipeline skew) -----------
            gbufs = {}

            def issue_batch(bi):
                if bi >= n_batches or bi in gbufs:
                    return
                e0 = bi * GBATCH
                g_n = min(GBATCH, L_tot - e0)
                w = g_n // 2
                gt = gpool.tile([128, GBATCH], cdt, tag="gbuf")
                nc.sync.dma_start(out=gt[:, :g_n], in_=gx_t[:, e0:e0 + g_n])
                et = eapool.tile([2 * EA, GBATCH // 2], cdt, tag="eab")
                nc.sync.dma_start(out=et[:, :w],
                                  in_=ea2[:, e0 // 2:e0 // 2 + w])
                ot = ohpool.tile([CHUNK, GBATCH], dt.float8e4, tag="ohb")
                nc.sync.dma_start(out=ot[:, :g_n], in_=oh_t[:, e0:e0 + g_n])
                gbufs[bi] = (gt, et, ot, w)

            issue_batch(0)
            issue_batch(1)
            # heavier consts load behind the first edge batches; none are
            # needed until the first scatter/node phase
            wna_s = load_const(wna[:], [D, D])
            wnb_s = load_const(wnb[:], [D, D])
            b1_s = load_const(b1[:], [D, 1], dt.float32)
            xt_s = load_const(xt_loc[:], [D, NPC])

            h_q = {}
            eh_q = {}
            ps2_q = {}
            xb_q = {}
            state = dict(abn=None)
            blk_start_of = np.repeat(blk_start, n_chunks)

            def do_mm2(s2, kk):
                (h, off), ps2 = h_q[s2], ps2_q[s2]
                ksl = slice(kk * CHUNK, (kk + 1) * CHUNK)
                hsl = slice(off + kk * CHUNK, off + (kk + 1) * CHUNK)
                nc.tensor.matmul(ps2[:, ksl], h[:, hsl], w2_s[:],
                                 start=True, stop=skip_bias2)
                if not skip_bias2:
                    nc.tensor.matmul(ps2[:, ksl], ones_s[:], b2r_s[:],
                                     start=False, stop=True)

            def flush_node(t_now):
                if state.get("pend") and (t_now is None
                                          or t_now >= state["pend"][0] + 3):
                    _, b, an_ps, agg_sb, xb = state.pop("pend")
                    node_mlp(b, an_ps, agg_sb, xb)

            def do_scatter(s3, kk):
                eh = eh_q[s3]
                t = (SUPER // CHUNK) * s3 + kk
                flush_node(t)
                b = int(blk_of_chunk[t])
                nb_c = int(n_chunks[b])
                rel = t - int(blk_start_of[t])
                # two alternating accumulators (A/B halves of one packed PSUM
                # tile) break the PSUM-RAW accumulate chain; third slice is
                # the node-MLP accumulator for this block
                if rel == 0:
                    # A | B | N packed in ONE PSUM bank.  Only this block's
                    # first matmul uses start=True (clears the bank's
                    # has_written bits); every later first-write to an
                    # untouched column region overwrites naturally.
                    abn_ps = psum.tile([128, 2 * BLK + D], dt.float32,
                                       tag="abn_ps")
                    state["abn"] = abn_ps
                    n_w = min(BLK, NPC - b * BLK)
                    xb = npool.tile([128, D], dt.float32, tag="xb")
                    nc.sync.dma_start(out=xb[:n_w, :],
                                      in_=x_loc[b * BLK:b * BLK + n_w, :])
                    xb_q[b] = xb
                abn_ps = state["abn"]
                p = rel % 2
                acc = abn_ps[:, p * BLK:(p + 1) * BLK]
                last_rel = nb_c - 1 - ((nb_c - 1 - p) % 2)
                bo, to = divmod(t, GBATCH // CHUNK)
                ot = gbufs[bo][2]
                ksl = slice(kk * CHUNK, (kk + 1) * CHUNK)
                nc.tensor.matmul(acc, eh[:, ksl],
                                 ot[:, to * CHUNK:(to + 1) * CHUNK],
                                 start=rel == 0, stop=rel == last_rel)
                if rel == nb_c - 1:
                    flush_node(None)   # never hold two pending blocks
                    agg_sb = npool.tile([128, BLK], cdt, tag="agg")
                    if nb_c >= 2:
                        # tensor ops may read only one PSUM operand: stage B
                        bsb = npool.tile([128, BLK], cdt, tag="bsb")
                        nc.vector.tensor_copy(bsb[:], abn_ps[:, BLK:2 * BLK])
                        nc.vector.tensor_add(agg_sb[:], abn_ps[:, 0:BLK],
                                             bsb[:])
                    else:
                        nc.vector.tensor_copy(agg_sb[:], abn_ps[:, 0:BLK])
                    # defer the node matmuls a few chunk slots so the PE
                    # never waits on the DVE merge
                    state["pend"] = (t, b, abn_ps, agg_sb, xb_q.pop(b))

            def mm1a(sx, ps1v):
                bi, k = divmod(sx, GBATCH // SUPER)
                gt = gbufs[bi][0]
                nc.tensor.matmul(ps1v, w1a_s[:],
                                 gt[:, k * SUPER:(k + 1) * SUPER],
                                 start=True, stop=False)

            def mm1b(sx, ps1v):
                bi, k = divmod(sx, GBATCH // SUPER)
                et, w = gbufs[bi][1], gbufs[bi][3]
                a, c0 = divmod(k * SUPER, w)
                nc.tensor.matmul(ps1v,
                                 w1b_s[32 * a:32 * a + 32, :],
                                 et[32 * a:32 * a + 32, c0:c0 + SUPER],
                                 start=False, stop=True)

            for it in range(n_sup + 2):
                s1, s2, s3 = it, it - 1, it - 2
                # mm1 runs on PAIRS of supers (even iterations) so the w1a/w1b
                # stationaries are reused and the ps1 accumulate chains of the
                # two supers hide each other's latency
                em1 = s1 < n_sup and s1 % 2 == 0
                pw = 2 if s1 + 1 < n_sup else 1
                em2 = 0 <= s2 < n_sup
                em3 = s3 >= 0
                if em1:
                    bi, k = divmod(s1, GBATCH // SUPER)
                    if k == 0:
                        issue_batch(bi + 1)
                    if k == 4:
                        issue_batch(bi + 2)
                    ps1 = psum.tile([128, 2 * SUPER], dt.float32, tag="ps1")
                    for j in range(pw):
                        mm1a(s1 + j, ps1[:, j * SUPER:(j + 1) * SUPER])
                if em2:
                    ps2 = psum.tile([128, SUPER], dt.float32, tag="ps2")
                    ps2_q[s2] = ps2
                    do_mm2(s2, 0)
                if em3:
                    do_scatter(s3, 0)
                if em2:
                    do_mm2(s2, 1)
                if em3:
                    do_scatter(s3, 1)
                if em1:
                    for j in range(pw):
                        mm1b(s1 + j, ps1[:, j * SUPER:(j + 1) * SUPER])
                    h = hpool.tile([128, 2 * SUPER], cdt, tag="h")
                    nc.scalar.activation(h[:, :pw * SUPER],
                                         ps1[:, :pw * SUPER], relu,
                                         bias=b1_s[:])
                    for j in range(pw):
                        h_q[s1 + j] = (h, j * SUPER)
                if em2:
                    do_mm2(s2, 2)
                if em3:
                    do_scatter(s3, 2)
                if em2:
                    do_mm2(s2, 3)
                if em3:
                    do_scatter(s3, 3)
                    eh_q.pop(s3)
                if em2:
                    h_q.pop(s2)
                    eh = epool.tile([128, SUPER], cdt, tag="eh")
                    if s2 % 8 == 0:
                        nc.scalar.activation(eh[:], ps2_q[s2][:], relu)
                    else:
                        nc.vector.tensor_scalar_max(eh[:], ps2_q[s2][:], 0.0)
                    ps2_q.pop(s2)
                    eh_q[s2] = eh
            flush_node(None)

    nc.compile()
    return nc


def _prepare(**inputs):
    x = np.ascontiguousarray(np.asarray(inputs["x"], F32))
    ei = np.asarray(inputs["edge_index"]).astype(np.int64)
    ea = np.ascontiguousarray(np.asarray(inputs["edge_attr"], F32))
    W_e1 = np.asarray(inputs["W_e1"], F32)
    b_e1 = np.asarray(inputs["b_e1"], F32)
    W_e2 = np.asarray(inputs["W_e2"], F32)
    b_e2 = np.asarray(inputs["b_e2"], F32)
    W_n = np.asarray(inputs["W_n"], F32)
    b_n = np.asarray(inputs["b_n"], F32)
    gamma = np.asarray(inputs["gamma"], F32)
    beta = np.asarray(inputs["beta"], F32)

    cnp = _np_cdt()
    dest, src = ei[0], ei[1]
    sched, per_core = _build_schedule(dest, src)
    sched["skip_bias2"] = bool(np.all(b_e2 == 0))
    sched["skip_biasn"] = bool(np.all(b_n == 0))
    sched["skip_affine"] = bool(np.all(gamma == 1) and np.all(beta == 0))
    nc = _build_bass(sched)

    import ml_dtypes
    L_tot = sched["L_tot"]
    T_tot = sched["T_tot"]
    ones_r = np.ones((1, 128), cnp)
    iota_d = np.arange(BLK, dtype=F32)
    gma = np.tile(gamma[None, :], (128, 1)).astype(F32)
    bta = np.tile(beta[None, :], (128, 1)).astype(F32)

    ea_z = np.concatenate([ea, np.zeros((1, EA), F32)], axis=0)  # -1 -> zeros
    x_cdt = x.astype(cnp)

    def pack_ea(ea_pad):
        """[L, 32] -> [64, L/2]: row 32a+d, col c = ea_pad[a*w + c, d] per
        GBATCH-edge batch of width w = g_n//2."""
        outs = []
        for e0 in range(0, L_tot, GBATCH):
            g_n = min(GBATCH, L_tot - e0)
            w = g_n // 2
            blk = ea_pad[e0:e0 + g_n].reshape(2, w, EA)
            outs.append(blk.transpose(0, 2, 1).reshape(2 * EA, w))
        return np.ascontiguousarray(np.concatenate(outs, axis=1))

    in_maps = []
    for c in range(NC):
        pc = per_core[c]
        gx = np.ascontiguousarray(x_cdt[pc["src"]].T)       # [128, L] bf16
        ea2 = pack_ea(ea_z[pc["ea_perm"]].astype(cnp))      # [64, L/2]
        dr = pc["dest_rel"].reshape(T_tot, CHUNK)           # [T, e]
        oh = (dr[:, :, None] == iota_d[None, None, :])      # [T, e, d]
        oh_t = np.ascontiguousarray(
            oh.transpose(1, 0, 2).reshape(CHUNK, L_tot)
        ).astype(ml_dtypes.float8_e4m3fn)
        xs = np.ascontiguousarray(x[sched["perm_nodes"][c]])
        in_maps.append({
            "gx_t": gx, "ea2": ea2, "oh_t": oh_t,
            "xt_loc": np.ascontiguousarray(xs.T.astype(cnp)),
            "x_loc": xs,
            "w1a": np.ascontiguousarray(W_e1[:D].astype(cnp)),
            "w1b2": np.ascontiguousarray(np.tile(W_e1[D:], (2, 1)).astype(cnp)),
            "w2": W_e2.astype(cnp),
            "wna": np.ascontiguousarray(W_n[:D].astype(cnp)),
            "wnb": np.ascontiguousarray(W_n[D:].astype(cnp)),
            "b1": b_e1[:, None].copy(),
            "b2r": b_e2[None, :].astype(cnp),
            "bnr": b_n[None, :].astype(cnp),
            "ones_r": ones_r, "gma": gma, "bta": bta,
        })
    return nc, in_maps, sched["perm_nodes"]


def kernel(**inputs) -> np.ndarray:
    nc, in_maps, perm_nodes = _prepare(**inputs)
    from concourse.bass_utils import run_bass_kernel_spmd
    res = run_bass_kernel_spmd(nc, in_maps, list(range(NC)))
    out = np.empty((N_NODES, D), np.float32)
    for c in range(NC):
        out[perm_nodes[c]] = np.asarray(res.results[c]["out"]).astype(np.float32)
    return out
